# revision 1
# baseline (speedup 1.0000x reference)
"""Bass/Trainium2 kernel for nn_GRUClassifier: 2-layer BiGRU + max-pool + MLP head.

Sharding: 8 cores = 4 batch groups x 2 L1-direction roles. Each core computes
BOTH L0 directions for its 16 sequences (duplicated across the role pair so no
cross-core exchange is needed), then one L1 direction, max-pool over time, and
the W1 partial of the classifier head. Host sums the two W1 partials per batch
group and applies relu + W2 (8.4 KFLOP per sample vs ~3.7 GFLOP on device).

All matmul operands fp16, accumulation fp32 in PSUM. Sequence reversal for the
backward direction is done by feeding the reversed token stream (host prep) so
every core runs the identical SPMD program; the only cross-order access is the
L1 "peer half" input projection, which is stored in produced order and read at
compile-time reversed offsets inside the unrolled recurrence.
"""
import os
import sys
import numpy as np

sys.path.insert(0, "/opt/trn_rl_repo")

B, T, E, H, V = 64, 256, 300, 512, 50000
EP = 384            # E padded to 3*128
G = 3 * H           # 1536 gate rows = 12 chunks of 128
BL = 16             # batch per core
NBLK = 8            # token blocks of 512 (= 32 steps * 16 batch)
SBLK = 16           # steps per xp/y block
NTOK = T * BL       # 4096

F16 = None          # set after imports in _build
F32 = None

_CACHE = {}


def _patch_drain():
    """walrus CoreV3 rejects CTRL (Drain) instructions with too many sem
    waits; split the tail-drain's waits across preceding sync nops."""
    from concourse import mybir
    from concourse.tile import TileContext
    from concourse.vector_clock import ScopedClock

    if getattr(TileContext, "_drain_patched", False):
        return
    MAXW = 1

    def _drain_and_barrier(self, tick_clock, wait_clock):
        drain_inst = self.nc.sync.drain()
        wait_clock.add_sem_waits(
            drain_inst.ins, ScopedClock({None: tick_clock.global_clock})
        )
        si = drain_inst.ins.sync_info
        if si is not None and si.on_wait and len(si.on_wait) > MAXW:
            waits = list(si.on_wait)
            si.on_wait = waits[:MAXW]
            for i in range(MAXW, len(waits), MAXW):
                nop = self.nc.sync.nop(nofuse=True, hint="drain_wait_split")
                nsi = nop.ins.sync_info
                if nsi is None:
                    nop.ins.sync_info = mybir.SyncInfo(
                        on_wait=waits[i : i + MAXW], on_update=[]
                    )
                else:
                    nsi.on_wait = waits[i : i + MAXW]
        self.nc.all_engine_barrier()
        assert self.sems is not None
        popped = self.nc._tile_sem_poison_stack.pop()
        assert popped is self._sem_poison
        self.nc.clear_and_free_semaphores(list(self.sems.allocated().values()))
        self.nc.all_engine_barrier()

    TileContext._drain_and_barrier = _drain_and_barrier
    TileContext._drain_patched = True


def _split_multiwaits(nc, mybir, maxw=1):
    """walrus CoreV2/V3 setupSyncWait rejects instructions with more than one
    sem wait; split extras onto preceding same-engine nops."""
    cnt = 0
    for fn in nc.m.functions:
        for bb in fn.blocks:
            insts = bb.instructions
            out = []
            changed = False
            for inst in insts:
                si = getattr(inst, "sync_info", None)
                eng = getattr(inst, "engine", None)
                if (
                    si is not None
                    and si.on_wait
                    and len(si.on_wait) > maxw
                    and eng is not None
                    and eng != mybir.EngineType.Unassigned
                ):
                    waits = list(si.on_wait)
                    for w in waits[:-maxw]:
                        nop = mybir.InstNoOp(
                            name=f"ws_nop_{cnt}", ins=[], outs=[]
                        )
                        cnt += 1
                        nop.engine = eng
                        nop.sync_info = mybir.SyncInfo(
                            on_wait=[w], on_update=[]
                        )
                        out.append(nop)
                    si.on_wait = waits[-maxw:]
                    changed = True
                out.append(inst)
            if changed:
                bb.instructions = out


def _build_nc():
    from concourse import bass, mybir
    from concourse.tile import TileContext

    _patch_drain()
    f16 = mybir.dt.float16
    f32 = mybir.dt.float32
    AF = mybir.ActivationFunctionType
    OP = mybir.AluOpType

    nc = bass.Bass(target_bir_lowering=False)

    def par(name, shape, dt=f16, out=False):
        return nc.declare_dram_parameter(name, list(shape), dt, isOutput=out)

    eT1 = par("eT1", [128, 3, NTOK])          # phase-1 embedded input (transposed)
    eT2 = par("eT2", [128, 3, NTOK])          # phase-2 (other direction's order)
    wih1 = par("wih1", [128, 3, G])           # L0 W_ih^T k-tiles, phase-1 dir
    wih2 = par("wih2", [128, 3, G])
    whh1 = par("whh1", [128, 4, G])           # L0 W_hh^T k-tiles
    whh2 = par("whh2", [128, 4, G])
    wa = par("wa", [128, 4, G])               # L1 W_ih^T, direct-source half
    wb = par("wb", [128, 4, G])               # L1 W_ih^T, reversed-source half
    whhL = par("whhL", [128, 4, G])
    bias1 = par("bias1", [128, 12], f32)      # xp bias per gate chunk (n: b_ih only)
    bias2 = par("bias2", [128, 12], f32)
    biasL = par("biasL", [128, 12], f32)
    nb1 = par("nb1", [128, 4], f32)           # b_hh n-gate chunks
    nb2 = par("nb2", [128, 4], f32)
    nbL = par("nbL", [128, 4], f32)
    w1h = par("w1h", [128, 4, 128])           # classifier W1 own-half^T k-tiles
    headout = par("headout", [128, 16], f32, out=True)

    xp1d = nc.dram_tensor("xp1d", [128, 12, NTOK], f16)
    xp2d = nc.dram_tensor("xp2d", [128, 12, NTOK], f16)
    xpad = nc.dram_tensor("xpad", [128, 12, NTOK], f16)
    xpbd = nc.dram_tensor("xpbd", [128, 12, NTOK], f16)
    y1d = nc.dram_tensor("y1d", [128, 4, NTOK], f16)
    y2d = nc.dram_tensor("y2d", [128, 4, NTOK], f16)

    with TileContext(nc) as tc:
        with (
            tc.tile_pool(name="wpool", bufs=1) as wp,
            tc.tile_pool(name="io", bufs=3) as io,
            tc.tile_pool(name="xpp", bufs=2) as xpp,
            tc.tile_pool(name="ew", bufs=2) as ew,
            tc.tile_pool(name="hp", bufs=2) as hp,
            tc.tile_pool(name="ps", bufs=2, space="PSUM") as ps,
            tc.tile_pool(name="psg", bufs=4, space="PSUM") as psg,
        ):
            # --- load all weights/biases into SBUF ---
            def load(p, shape, dt=f16):
                t = wp.tile(list(shape), dt, tag=p.name + "_sb")
                nc.sync.dma_start(out=t[:], in_=p[:])
                return t

            wih1_s = load(wih1, [128, 3, G])
            wih2_s = load(wih2, [128, 3, G])
            whh1_s = load(whh1, [128, 4, G])
            whh2_s = load(whh2, [128, 4, G])
            wa_s = load(wa, [128, 4, G])
            wb_s = load(wb, [128, 4, G])
            whhL_s = load(whhL, [128, 4, G])
            bias1_s = load(bias1, [128, 12], f32)
            bias2_s = load(bias2, [128, 12], f32)
            biasL_s = load(biasL, [128, 12], f32)
            nb1_s = load(nb1, [128, 4], f32)
            nb2_s = load(nb2, [128, 4], f32)
            nbL_s = load(nbL, [128, 4], f32)
            w1h_s = load(w1h, [128, 4, 128])

            def xp_gemm_blk(blk, src_dram, w_sb, kt, bias_sb, dst_dram):
                    sl = slice(blk * 512, (blk + 1) * 512)
                    et = io.tile([128, kt, 512], f16, tag="xg_in")
                    nc.sync.dma_start(out=et[:], in_=src_dram[:, :, sl])
                    for m in range(12):
                        p = ps.tile([128, 512], f32, tag="gemm_ps")
                        for k in range(kt):
                            nc.tensor.matmul(
                                p[:],
                                w_sb[:, k, m * 128 : (m + 1) * 128],
                                et[:, k, :],
                                start=(k == 0),
                                stop=(k == kt - 1),
                            )
                        xs = io.tile([128, 512], f16, tag="xg_out")
                        nc.scalar.activation(
                            xs[:], p[:], AF.Identity, bias=bias_sb[:, m : m + 1]
                        )
                        nc.sync.dma_start(out=dst_dram[:, m, sl], in_=xs[:])

            def xp_gemm(src_dram, w_sb, kt, bias_sb, dst_dram, tag):
                for blk in range(NBLK):
                    xp_gemm_blk(blk, src_dram, w_sb, kt, bias_sb, dst_dram)

            def recurrence(whh_sb, xp_dram, nb_sb, y_dram=None, xpb_dram=None,
                           pooled=None, side=None, tag="rc"):
                h = hp.tile([128, 4, 16], f16, tag="rc_h")
                nc.vector.memset(h[:], 0.0)
                yb = None
                for t in range(T):
                    if side and t in side:
                        for fn in side[t]:
                            fn()
                    blk, v = t // SBLK, t % SBLK
                    vs = slice(v * 16, (v + 1) * 16)
                    rv = SBLK - 1 - v
                    rvs = slice(rv * 16, (rv + 1) * 16)
                    if v == 0:
                        sl = slice(blk * 256, (blk + 1) * 256)
                        xpt = xpp.tile([128, 12, 256], f16, tag="rc_xpt")
                        nc.sync.dma_start(out=xpt[:], in_=xp_dram[:, :, sl])
                        if xpb_dram is not None:
                            rb = (T // SBLK) - 1 - blk
                            rsl = slice(rb * 256, (rb + 1) * 256)
                            xbt = xpp.tile([128, 12, 256], f16, tag="rc_xbt")
                            nc.sync.dma_start(out=xbt[:], in_=xpb_dram[:, :, rsl])
                        if y_dram is not None:
                            yb = io.tile([128, 4, 256], f16, tag="rc_yb")
                    pst = psg.tile([128, 12, 16], f32, tag="rc_ps")
                    for m in range(12):
                        out = pst[:, m, :]
                        for k in range(4):
                            nc.tensor.matmul(
                                out,
                                whh_sb[:, k, m * 128 : (m + 1) * 128],
                                h[:, k, :],
                                start=(k == 0),
                                stop=(k == 3),
                            )
                    def gsum(lo, hi, otag):
                        o = ew.tile([128, 4, 16], f32, tag=otag)
                        nc.vector.scalar_tensor_tensor(
                            out=o[:], in0=pst[:, lo:hi, :], scalar=1.0,
                            in1=xpt[:, lo:hi, vs], op0=OP.mult, op1=OP.add,
                        )
                        if xpb_dram is not None:
                            nc.vector.scalar_tensor_tensor(
                                out=o[:], in0=o[:], scalar=1.0,
                                in1=xbt[:, lo:hi, rvs], op0=OP.mult, op1=OP.add,
                            )
                        return o
                    tr = gsum(0, 4, "rc_tr")
                    r = ew.tile([128, 4, 16], f16, tag="rc_r")
                    nc.scalar.activation(r[:], tr[:], AF.Sigmoid)
                    tz = gsum(4, 8, "rc_tz")
                    z = ew.tile([128, 4, 16], f16, tag="rc_z")
                    zb = ew.tile([128, 4, 16], f16, tag="rc_zb")
                    nc.scalar.activation(z[:], tz[:], AF.Sigmoid)
                    nc.scalar.activation(zb[:], tz[:], AF.Sigmoid, scale=-1.0)
                    u = ew.tile([128, 4, 16], f32, tag="rc_u")
                    for q in range(4):
                        nc.vector.scalar_tensor_tensor(
                            out=u[:, q, :], in0=pst[:, 8 + q, :],
                            scalar=nb_sb[:, q : q + 1], in1=r[:, q, :],
                            op0=OP.add, op1=OP.mult,
                        )
                    tn = ew.tile([128, 4, 16], f32, tag="rc_tn")
                    nc.vector.scalar_tensor_tensor(
                        out=tn[:], in0=u[:], scalar=1.0,
                        in1=xpt[:, 8:12, vs], op0=OP.mult, op1=OP.add,
                    )
                    if xpb_dram is not None:
                        nc.vector.scalar_tensor_tensor(
                            out=tn[:], in0=tn[:], scalar=1.0,
                            in1=xbt[:, 8:12, rvs], op0=OP.mult, op1=OP.add,
                        )
                    n = ew.tile([128, 4, 16], f16, tag="rc_n")
                    nc.scalar.activation(n[:], tn[:], AF.Tanh)
                    a = ew.tile([128, 4, 16], f16, tag="rc_a")
                    nc.vector.scalar_tensor_tensor(
                        out=a[:], in0=z[:], scalar=1.0, in1=h[:],
                        op0=OP.mult, op1=OP.mult,
                    )
                    b2 = ew.tile([128, 4, 16], f16, tag="rc_b2")
                    nc.vector.scalar_tensor_tensor(
                        out=b2[:], in0=zb[:], scalar=1.0, in1=n[:],
                        op0=OP.mult, op1=OP.mult,
                    )
                    hn = hp.tile([128, 4, 16], f16, tag="rc_h")
                    nc.vector.scalar_tensor_tensor(
                        out=hn[:], in0=a[:], scalar=1.0, in1=b2[:],
                        op0=OP.mult, op1=OP.add,
                    )
                    if pooled is not None:
                        nc.vector.scalar_tensor_tensor(
                            out=pooled[:], in0=pooled[:], scalar=1.0, in1=hn[:],
                            op0=OP.mult, op1=OP.max,
                        )
                    if y_dram is not None:
                        nc.vector.tensor_copy(out=yb[:, :, vs], in_=hn[:])
                        if v == SBLK - 1:
                            sl = slice(blk * 256, (blk + 1) * 256)
                            nc.sync.dma_start(out=y_dram[:, :, sl], in_=yb[:])
                    h = hn

            zb12 = wp.tile([128, 12], f32, tag="zbias")
            nc.vector.memset(zb12[:], 0.0)
            # ---- phase A: L0 phase-1 input projection ----
            xp_gemm(eT1, wih1_s, 3, bias1_s, xp1d, "xg1")
            # ---- L0 recurrence 1, with phase-B GEMM blocks interleaved ----
            sideB = {
                b * 32: [
                    (lambda bb: lambda: xp_gemm_blk(
                        bb, eT2, wih2_s, 3, bias2_s, xp2d))(b)
                ]
                for b in range(NBLK)
            }
            recurrence(whh1_s, xp1d, nb1_s, y_dram=y1d, side=sideB, tag="r1")
            # ---- L0 recurrence 2, with L1 projection blocks interleaved ----
            sideE = {}
            for k in range(1, 8):
                sideE[32 * k] = [
                    (lambda bb: lambda: xp_gemm_blk(
                        bb, y1d, wa_s, 4, biasL_s, xpad))(k - 1),
                    (lambda bb: lambda: xp_gemm_blk(
                        bb, y2d, wb_s, 4, zb12, xpbd))(k - 1),
                ]
            recurrence(whh2_s, xp2d, nb2_s, y_dram=y2d, side=sideE, tag="r2")
            xp_gemm_blk(7, y1d, wa_s, 4, biasL_s, xpad)
            xp_gemm_blk(7, y2d, wb_s, 4, zb12, xpbd)
            # ---- L1 recurrence with on-the-fly max pool ----
            pooled = wp.tile([128, 4, 16], f16, tag="pooled")
            nc.vector.memset(pooled[:], -60000.0)
            recurrence(whhL_s, xpad, nbL_s, xpb_dram=xpbd, pooled=pooled, tag="rL")
            # ---- head partial: W1_half @ pooled ----
            hd = ps.tile([128, 16], f32, tag="gemm_ps")
            for k in range(4):
                nc.tensor.matmul(
                    hd[:], w1h_s[:, k, :], pooled[:, k, :],
                    start=(k == 0), stop=(k == 3),
                )
            ho = io.tile([128, 16], f32, tag="head_sb")
            nc.vector.tensor_copy(out=ho[:], in_=hd[:])
            nc.sync.dma_start(out=headout[:], in_=ho[:])

    _split_multiwaits(nc, mybir)
    try:
        ents = getattr(tc, "_perfetto_entries", None)
        span = None
        if ents:
            # (tile_name, allocated_time, freed_time, space, bytes, addr, tag)
            starts = [e[1] for e in ents if e[1] is not None]
            ends = [e[2] if e[2] is not None else e[1] for e in ents]
            if starts and ends:
                span = int(max(ends) - min(starts))
        _CACHE["model_ns"] = span
    except Exception:
        _CACHE["model_ns"] = None
    return nc


def _prep_core_inputs(inputs, g, role):
    """Host-side sharding/layout prep for core (batch group g, role)."""
    f16 = np.float16
    x = np.asarray(inputs["x"]).astype(np.int64)
    emb = np.asarray(inputs["emb"], dtype=np.float32)
    embp = np.zeros((V, EP), dtype=np.float32)
    embp[:, :E] = emb

    xg = x[g * BL : (g + 1) * BL]                     # [16, 256]
    e = embp[xg]                                      # [16, 256, 384]
    # eT[:, t*16+b] = e[b, t]  -> [384, 4096]
    eT_f = np.ascontiguousarray(e.transpose(2, 1, 0).reshape(EP, NTOK))
    er = e[:, ::-1, :]
    eT_r = np.ascontiguousarray(er.transpose(2, 1, 0).reshape(EP, NTOK))

    def ktile(wT, kt):   # [K, G'] -> [128, kt, G']
        Kd, Gd = wT.shape
        assert Kd == kt * 128
        return np.ascontiguousarray(
            wT.reshape(kt, 128, Gd).transpose(1, 0, 2)
        ).astype(f16)

    def e3(eT):          # [384, NTOK] -> [128, 3, NTOK]
        return np.ascontiguousarray(
            eT.reshape(3, 128, NTOK).transpose(1, 0, 2)
        ).astype(f16)

    def biascols(b_ih, b_hh):
        bv = b_ih.copy()
        bv[: 2 * H] += b_hh[: 2 * H]                  # r,z get both biases
        cols = np.ascontiguousarray(bv.reshape(12, 128).T).astype(np.float32)
        nb = np.ascontiguousarray(
            b_hh[2 * H :].reshape(4, 128).T
        ).astype(np.float32)
        return cols, nb

    w_ih0 = np.asarray(inputs["w_ih0"], dtype=np.float32)
    w_hh0 = np.asarray(inputs["w_hh0"], dtype=np.float32)
    b_ih0 = np.asarray(inputs["b_ih0"], dtype=np.float32)
    b_hh0 = np.asarray(inputs["b_hh0"], dtype=np.float32)
    w_ih1 = np.asarray(inputs["w_ih1"], dtype=np.float32)
    w_hh1 = np.asarray(inputs["w_hh1"], dtype=np.float32)
    b_ih1 = np.asarray(inputs["b_ih1"], dtype=np.float32)
    b_hh1 = np.asarray(inputs["b_hh1"], dtype=np.float32)
    w1 = np.asarray(inputs["w1"], dtype=np.float32)

    d1, d2 = (0, 1) if role == 0 else (1, 0)          # phase-1 dir, phase-2 dir
    dL = role                                          # L1 direction
    own_half = slice(0, H) if role == 0 else slice(H, 2 * H)
    oth_half = slice(H, 2 * H) if role == 0 else slice(0, H)

    def wihT(d):
        w = np.zeros((G, EP), dtype=np.float32)
        w[:, :E] = w_ih0[d]
        return ktile(w.T, 3)

    b1c, n1c = biascols(b_ih0[d1], b_hh0[d1])
    b2c, n2c = biascols(b_ih0[d2], b_hh0[d2])
    bLc, nLc = biascols(b_ih1[dL], b_hh1[dL])

    m = {
        "eT1": e3(eT_f if role == 0 else eT_r),
        "eT2": e3(eT_r if role == 0 else eT_f),
        "wih1": wihT(d1),
        "wih2": wihT(d2),
        "whh1": ktile(w_hh0[d1].T, 4),
        "whh2": ktile(w_hh0[d2].T, 4),
        "wa": ktile(w_ih1[dL][:, own_half].T, 4),
        "wb": ktile(w_ih1[dL][:, oth_half].T, 4),
        "whhL": ktile(w_hh1[dL].T, 4),
        "bias1": b1c, "bias2": b2c, "biasL": bLc,
        "nb1": n1c, "nb2": n2c, "nbL": nLc,
        "w1h": ktile(w1[:, own_half].T, 4),
    }
    return m


def kernel(**inputs) -> np.ndarray:
    from concourse.bass_utils import run_bass_kernel_spmd

    if "nc" not in _CACHE:
        _CACHE["nc"] = _build_nc()
    nc = _CACHE["nc"]

    core_ids = list(range(8))
    in_maps = []
    for c in core_ids:
        g, role = c % 4, c // 4
        in_maps.append(_prep_core_inputs(inputs, g, role))

    res = run_bass_kernel_spmd(nc, in_maps, core_ids)
    _CACHE["last_res"] = res

    b1 = np.asarray(inputs["b1"], dtype=np.float32)
    w2 = np.asarray(inputs["w2"], dtype=np.float32)
    b2 = np.asarray(inputs["b2"], dtype=np.float32)
    out = np.zeros((B, 2), dtype=np.float32)
    for g in range(4):
        p = (
            res.results[g]["headout"].astype(np.float32)
            + res.results[g + 4]["headout"].astype(np.float32)
        )                                              # [128 hid, 16 batch]
        hid = np.maximum(p + b1[:, None], 0.0)
        logits = w2 @ hid + b2[:, None]                # [2, 16]
        out[g * BL : (g + 1) * BL] = logits.T
    return out



# revision 25
# speedup vs baseline: 2.1960x; 2.1960x over previous
"""Bass/Trainium2 kernel for nn_GRUClassifier: 2-layer BiGRU + max-pool + MLP head.

Sharding: 8 cores = 4 batch groups x 2 L1-direction roles (as baseline).
Each core computes BOTH L0 directions for its 16 sequences, then one L1
direction, max-pool, and the W1 partial; host sums role-pair partials and
applies relu + W2.

Perf design (cost-model driven; the recurrence is dependency-chain-bound):
- The two L0 recurrences run INTERLEAVED step-by-step (independent chains)
  so engine latency of one chain hides under the other.
- Per-step op count: 5 DVE + 2 Act:
  * xp and the n-gate b_hh bias are injected into PSUM with identity
    matmuls (PE is nearly free), removing the gate-sum DVE ops.
    Accumulation groups are kept CONTIGUOUS per PSUM region (interleaved
    start-groups within a bank corrupt accumulation).
  * r,z sigmoids fused into one activation over [128,8,16]; separate PSUM
    tiles for rz vs n gates so the sigmoid does not wait on n matmuls
    (Tile tracks deps at whole-tile granularity).
  * h update: h = z*h + (1-z)*n with q=z*h computed off-chain and
    p=(z-1)*(-n) from the negated tanh output (z kept f32 to avoid
    cancellation in 1-z).
- xp blocks live entirely in SBUF ring tiles: the input-projection GEMMs
  write them directly (no DRAM roundtrip, no DMA races), sliced into small
  items paced between recurrence steps.
- y (L0 outputs) stay SBUF-resident; the backward chain writes y in
  position-reversed slots so BOTH halves of the L1 input GEMM accumulate
  in one PSUM pass (single xpL stream; no per-step peer adds).
"""
import os
import sys
import numpy as np

sys.path.insert(0, "/opt/trn_rl_repo")

B, T, E, H, V = 64, 256, 300, 512, 50000
EP = 384            # E padded to 3*128
G = 3 * H           # 1536 gate rows = 12 chunks of 128
BL = 16             # batch per core
NTOK = T * BL       # 4096
NBLK = 16           # 256-col xp blocks (16 steps each)

_CACHE = {}


def _patch_drain():
    """walrus CoreV3 rejects CTRL (Drain) instructions with too many sem
    waits; split the tail-drain's waits across preceding sync nops."""
    from concourse import mybir
    from concourse.tile import TileContext
    from concourse.vector_clock import ScopedClock

    if getattr(TileContext, "_drain_patched", False):
        return
    MAXW = 1

    def _drain_and_barrier(self, tick_clock, wait_clock):
        drain_inst = self.nc.sync.drain()
        wait_clock.add_sem_waits(
            drain_inst.ins, ScopedClock({None: tick_clock.global_clock})
        )
        si = drain_inst.ins.sync_info
        if si is not None and si.on_wait and len(si.on_wait) > MAXW:
            waits = list(si.on_wait)
            si.on_wait = waits[:MAXW]
            for i in range(MAXW, len(waits), MAXW):
                nop = self.nc.sync.nop(nofuse=True, hint="drain_wait_split")
                nsi = nop.ins.sync_info
                if nsi is None:
                    nop.ins.sync_info = mybir.SyncInfo(
                        on_wait=waits[i : i + MAXW], on_update=[]
                    )
                else:
                    nsi.on_wait = waits[i : i + MAXW]
        self.nc.all_engine_barrier()
        assert self.sems is not None
        popped = self.nc._tile_sem_poison_stack.pop()
        assert popped is self._sem_poison
        self.nc.clear_and_free_semaphores(list(self.sems.allocated().values()))
        self.nc.all_engine_barrier()

    TileContext._drain_and_barrier = _drain_and_barrier
    TileContext._drain_patched = True


def _split_multiwaits(nc, mybir, maxw=1):
    """walrus CoreV2/V3 setupSyncWait rejects instructions with more than one
    sem wait; split extras onto preceding same-engine nops."""
    cnt = 0
    for fn in nc.m.functions:
        for bb in fn.blocks:
            insts = bb.instructions
            out = []
            changed = False
            for inst in insts:
                si = getattr(inst, "sync_info", None)
                eng = getattr(inst, "engine", None)
                if (
                    si is not None
                    and si.on_wait
                    and len(si.on_wait) > maxw
                    and eng is not None
                    and eng != mybir.EngineType.Unassigned
                ):
                    waits = list(si.on_wait)
                    for w in waits[:-maxw]:
                        nop = mybir.InstNoOp(
                            name=f"ws_nop_{cnt}", ins=[], outs=[]
                        )
                        cnt += 1
                        nop.engine = eng
                        nop.sync_info = mybir.SyncInfo(
                            on_wait=[w], on_update=[]
                        )
                        out.append(nop)
                    si.on_wait = waits[-maxw:]
                    changed = True
                out.append(inst)
            if changed:
                bb.instructions = out


def _build_nc():
    from concourse import bass, mybir
    from concourse.tile import TileContext

    _patch_drain()
    f16 = mybir.dt.float16
    f32 = mybir.dt.float32
    AF = mybir.ActivationFunctionType
    OP = mybir.AluOpType

    nc = bass.Bass(target_bir_lowering=False)

    def par(name, shape, dt=f16, out=False):
        return nc.declare_dram_parameter(name, list(shape), dt, isOutput=out)

    eT1 = par("eT1", [128, 3, NTOK])
    eT2 = par("eT2", [128, 3, NTOK])
    wih1 = par("wih1", [128, 3, G])
    wih2 = par("wih2", [128, 3, G])
    whh1 = par("whh1", [128, 4, G])
    whh2 = par("whh2", [128, 4, G])
    wa = par("wa", [128, 4, G])
    wb = par("wb", [128, 4, G])
    whhL = par("whhL", [128, 4, G])
    biasr1 = par("biasr1", [1, 12, 128])
    biasr2 = par("biasr2", [1, 12, 128])
    biasrL = par("biasrL", [1, 12, 128])
    ones = par("ones", [1, 256])
    nbc1 = par("nbc1", [128, 4, 16])
    nbc2 = par("nbc2", [128, 4, 16])
    nbcL = par("nbcL", [128, 4, 16])
    ident = par("ident", [128, 128])
    w1h = par("w1h", [128, 4, 128])
    headout = par("headout", [128, 16], f32, out=True)
    DBG = os.environ.get("GRU_DEBUG_DUMP") == "1"
    if DBG:
        y1o = par("y1o", [128, 4, NTOK], out=True)
        y2o = par("y2o", [128, 4, NTOK], out=True)
        pooledo = par("pooledo", [128, 4, 16], out=True)

    with TileContext(nc) as tc:
        with (
            tc.tile_pool(name="wpool", bufs=1) as wp,
            tc.tile_pool(name="io", bufs=3) as io,
            tc.tile_pool(name="xpb", bufs=6) as xpb,
            tc.tile_pool(name="ew", bufs=2) as ew,
            tc.tile_pool(name="hp", bufs=2) as hp,
            tc.tile_pool(name="ps", bufs=2, space="PSUM") as ps,
            tc.tile_pool(name="psg", bufs=3, space="PSUM") as psg,
        ):
            def load(p, shape, dt=f16):
                t = wp.tile(list(shape), dt, tag=p.name + "_sb")
                nc.sync.dma_start(out=t[:], in_=p[:])
                return t

            # phase-A-critical weights first (SP DMA queue is in-order)
            wih1_s = load(wih1, [128, 3, G])
            biasr1_s = load(biasr1, [1, 12, 128])
            wih2_s = load(wih2, [128, 3, G])
            biasr2_s = load(biasr2, [1, 12, 128])
            ones_s = load(ones, [1, 256])
            ident_s = load(ident, [128, 128])
            nbc1_s = load(nbc1, [128, 4, 16])
            nbc2_s = load(nbc2, [128, 4, 16])
            whh1_s = load(whh1, [128, 4, G])
            whh2_s = load(whh2, [128, 4, G])

            def load_late(p, shape, dt=f16):
                # phase-B weights ride the idle GPSIMD DGE queue
                t = wp.tile(list(shape), dt, tag=p.name + "_sb")
                nc.gpsimd.dma_start(out=t[:], in_=p[:])
                return t

            wa_s = load_late(wa, [128, 4, G])
            wb_s = load_late(wb, [128, 4, G])
            whhL_s = load_late(whhL, [128, 4, G])
            biasrL_s = load_late(biasrL, [1, 12, 128])
            nbcL_s = load_late(nbcL, [128, 4, 16])
            w1h_s = load_late(w1h, [128, 4, 128])

            y1s = wp.tile([128, 4, NTOK], f16, tag="y1s")
            y2s = wp.tile([128, 4, NTOK], f16, tag="y2s")
            h0 = wp.tile([128, 4, 16], f16, tag="h0")
            nc.vector.memset(h0[:], 0.0)
            pooled = wp.tile([128, 4, 16], f16, tag="pooled")
            nc.vector.memset(pooled[:], -60000.0)

            ring = {}

            # ---------- GEMM item generators (finely sliced side work) ----
            # Each 256-col xp block is computed straight into an SBUF ring
            # tile; items are individually small so they pace between steps.
            # Bias is folded into the GEMM via a K=1 bias-row matmul, so the
            # PSUM->SBUF downcast is a bias-free Copy fused over 2 m-chunks
            # (fewer, Act-only fin ops keep the DVE queue clear for chains).
            def gemm_items_A(cid, eT_dram, w_sb, biasr_sb, blk):
                sl = slice(blk * 256, (blk + 1) * 256)
                st = {}
                items = []

                def open_():
                    ring[(cid, blk)] = xpb.tile(
                        [128, 12, 256], f16, tag="xpb", name="xpblk")
                    et = io.tile([128, 3, 256], f16, tag="et_in")
                    nc.sync.dma_start(out=et[:], in_=eT_dram[:, :, sl])
                    st["et"] = et

                items.append(open_)
                for m2 in range(6):
                    def mk(m2):
                        def mmsa():
                            p = ps.tile([128, 2, 256], f32, tag="gps")
                            st[m2] = p
                            m = 2 * m2
                            for k in range(3):
                                nc.tensor.matmul(
                                    p[:, 0, :],
                                    w_sb[:, k, m * 128 : (m + 1) * 128],
                                    st["et"][:, k, :],
                                    start=(k == 0), stop=False,
                                )
                            nc.tensor.matmul(
                                p[:, 0, :], biasr_sb[0:1, m, :],
                                ones_s[0:1, :], start=False, stop=True,
                            )

                        def mmsb():
                            p = st[m2]
                            m = 2 * m2 + 1
                            for k in range(3):
                                nc.tensor.matmul(
                                    p[:, 1, :],
                                    w_sb[:, k, m * 128 : (m + 1) * 128],
                                    st["et"][:, k, :],
                                    start=(k == 0), stop=False,
                                )
                            nc.tensor.matmul(
                                p[:, 1, :], biasr_sb[0:1, m, :],
                                ones_s[0:1, :], start=False, stop=True,
                            )

                        def fin():
                            p = st.pop(m2)
                            xb = ring[(cid, blk)]
                            nc.scalar.activation(
                                xb[:, 2 * m2 : 2 * m2 + 2, :], p[:], AF.Copy,
                            )

                        return mmsa, mmsb, fin

                    items += list(mk(m2))
                return items

            def gemm_items_L(blk):
                sl = slice(blk * 256, (blk + 1) * 256)
                st = {}
                items = []

                def open_():
                    ring[("L", blk)] = xpb.tile(
                        [128, 12, 256], f16, tag="xpb", name="xpblk")

                items.append(open_)
                for m2 in range(6):
                    def mk(m2):
                        def half(m, pi, p):
                            for k in range(4):
                                nc.tensor.matmul(
                                    p[:, pi, :],
                                    wa_s[:, k, m * 128 : (m + 1) * 128],
                                    y1s[:, k, sl],
                                    start=(k == 0), stop=False,
                                )
                            for k in range(4):
                                nc.tensor.matmul(
                                    p[:, pi, :],
                                    wb_s[:, k, m * 128 : (m + 1) * 128],
                                    y2s[:, k, sl],
                                    start=False, stop=False,
                                )
                            nc.tensor.matmul(
                                p[:, pi, :], biasrL_s[0:1, m, :],
                                ones_s[0:1, :], start=False, stop=True,
                            )

                        def mmsa():
                            p = ps.tile([128, 2, 256], f32, tag="gps")
                            st[m2] = p
                            half(2 * m2, 0, p)

                        def mmsb():
                            half(2 * m2 + 1, 1, st[m2])

                        def fin():
                            p = st.pop(m2)
                            xb = ring[("L", blk)]
                            nc.scalar.activation(
                                xb[:, 2 * m2 : 2 * m2 + 2, :], p[:], AF.Copy,
                            )

                        return mmsa, mmsb, fin

                    items += list(mk(m2))
                return items

            # ---------- recurrence chain -----------------------------------
            class Chain:
                def __init__(self, cid, whh_sb, nbc_sb,
                             y=None, rev=False, use_pool=False):
                    self.cid = cid
                    self.whh = whh_sb
                    self.nbc = nbc_sb
                    self.y = y
                    self.rev = rev
                    self.use_pool = use_pool
                    self.cur = None
                    self.hprev = h0

                def _col(self, t):
                    c = (T - 1 - t) if self.rev else t
                    return slice(c * 16, (c + 1) * 16)

                def step(self, t):
                    v = t % 16
                    if v == 0:
                        self.cur = ring.pop((self.cid, t // 16))
                    cur = self.cur
                    vs = slice(v * 16, (v + 1) * 16)
                    prz = psg.tile([128, 8, 16], f32, tag="pstrz")
                    pn = psg.tile([128, 4, 16], f32, tag="pstn")
                    hp_ = self.hprev

                    def hrhs(k):
                        return (
                            hp_[:, k, :] if not isinstance(hp_, tuple)
                            else hp_[0][:, k, hp_[1]]
                        )

                    for m in range(12):
                        dst = prz[:, m, :] if m < 8 else pn[:, m - 8, :]
                        src = (cur[:, m, vs] if m < 8
                               else self.nbc[:, m - 8, :])
                        nc.tensor.matmul(
                            dst, ident_s[:], src, start=True, stop=False,
                        )
                        for k in range(4):
                            nc.tensor.matmul(
                                dst,
                                self.whh[:, k, m * 128 : (m + 1) * 128],
                                hrhs(k),
                                start=False,
                                stop=(k == 3),
                            )
                    rz = ew.tile([128, 8, 16], f32, tag=f"rz{self.cid}")
                    nc.scalar.activation(rz[:], prz[:], AF.Sigmoid)
                    u = ew.tile([128, 4, 16], f32, tag=f"u{self.cid}")
                    nc.vector.scalar_tensor_tensor(
                        out=u[:], in0=pn[:], scalar=1.0,
                        in1=rz[:, 0:4, :], op0=OP.mult, op1=OP.mult,
                    )
                    tn = ew.tile([128, 4, 16], f32, tag=f"tn{self.cid}")
                    nc.vector.scalar_tensor_tensor(
                        out=tn[:], in0=u[:], scalar=1.0,
                        in1=cur[:, 8:12, vs], op0=OP.mult, op1=OP.add,
                    )
                    nn = ew.tile([128, 4, 16], f16, tag=f"nn{self.cid}")
                    nc.scalar.activation(nn[:], tn[:], AF.Tanh, scale=-1.0)
                    q = ew.tile([128, 4, 16], f16, tag=f"q{self.cid}")
                    nc.vector.scalar_tensor_tensor(
                        out=q[:], in0=rz[:, 4:8, :], scalar=1.0,
                        in1=hp_[:] if not isinstance(hp_, tuple)
                        else hp_[0][:, :, hp_[1]],
                        op0=OP.mult, op1=OP.mult,
                    )
                    p = ew.tile([128, 4, 16], f16, tag=f"p{self.cid}")
                    nc.vector.scalar_tensor_tensor(
                        out=p[:], in0=rz[:, 4:8, :], scalar=1.0,
                        in1=nn[:], op0=OP.subtract, op1=OP.mult,
                    )
                    if self.y is not None:
                        cs = self._col(t)
                        nc.vector.scalar_tensor_tensor(
                            out=self.y[:, :, cs], in0=q[:], scalar=1.0,
                            in1=p[:], op0=OP.mult, op1=OP.add,
                        )
                        self.hprev = (self.y, cs)
                    else:
                        hn = hp.tile([128, 4, 16], f16, tag="hL")
                        nc.vector.scalar_tensor_tensor(
                            out=hn[:], in0=q[:], scalar=1.0,
                            in1=p[:], op0=OP.mult, op1=OP.add,
                        )
                        if self.use_pool:
                            nc.vector.scalar_tensor_tensor(
                                out=pooled[:], in0=hn[:], scalar=1.0,
                                in1=pooled[:], op0=OP.mult, op1=OP.max,
                            )
                        self.hprev = hn

            # ---------- item pacing ---------------------------------------
            def paced(items, t0, t1):
                n = len(items)
                w = max(1, t1 - t0)
                return [(t0 + (i * w) // n, fn) for i, fn in enumerate(items)]

            def run_phase(chains, sched):
                sched = sorted(sched, key=lambda x: x[0])
                si = 0
                for t in range(T):
                    while si < len(sched) and sched[si][0] <= t:
                        sched[si][1]()
                        si += 1
                    for c in chains:
                        c.step(t)
                while si < len(sched):
                    sched[si][1]()
                    si += 1

            # ---------- phase A: L0 fwd + bwd interleaved -----------------
            for fn in gemm_items_A("A", eT1, wih1_s, biasr1_s, 0):
                fn()
            for fn in gemm_items_A("B", eT2, wih2_s, biasr2_s, 0):
                fn()
            chA = Chain("A", whh1_s, nbc1_s, y=y1s, rev=False)
            chB = Chain("B", whh2_s, nbc2_s, y=y2s, rev=True)
            schedA = []
            for c in range(1, NBLK):
                schedA += paced(
                    gemm_items_A("A", eT1, wih1_s, biasr1_s, c)
                    + gemm_items_A("B", eT2, wih2_s, biasr2_s, c),
                    max(0, 16 * c - 44), 16 * c - 8,
                )
            run_phase([chA, chB], schedA)

            # ---------- phase B: L1 (merged xpL GEMM) ---------------------
            for fn in gemm_items_L(0):
                fn()
            chL = Chain("L", whhL_s, nbcL_s, use_pool=True)
            schedB = []
            for c in range(1, NBLK):
                schedB += paced(
                    gemm_items_L(c), max(0, 16 * c - 44), 16 * c - 8
                )
            run_phase([chL], schedB)

            # ---------- head ----------------------------------------------
            hd = psg.tile([128, 4, 16], f32, tag="pstn")
            for k in range(4):
                nc.tensor.matmul(
                    hd[:, 0, :], w1h_s[:, k, :], pooled[:, k, :],
                    start=(k == 0), stop=(k == 3),
                )
            ho = io.tile([128, 16], f32, tag="ho")
            nc.vector.tensor_copy(out=ho[:], in_=hd[:, 0, :])
            nc.sync.dma_start(out=headout[:], in_=ho[:])
            if DBG:
                nc.sync.dma_start(out=y1o[:], in_=y1s[:])
                nc.sync.dma_start(out=y2o[:], in_=y2s[:])
                nc.sync.dma_start(out=pooledo[:], in_=pooled[:])

    _split_multiwaits(nc, mybir)
    try:
        ents = getattr(tc, "_perfetto_entries", None)
        span = None
        if ents:
            starts = [e[1] for e in ents if e[1] is not None]
            ends = [e[2] if e[2] is not None else e[1] for e in ents]
            if starts and ends:
                span = int(max(ends) - min(starts))
        _CACHE["model_ns"] = span
    except Exception:
        _CACHE["model_ns"] = None
    return nc


def _prep_core_inputs(inputs, g, role):
    """Host-side sharding/layout prep for core (batch group g, role)."""
    f16 = np.float16
    x = np.asarray(inputs["x"]).astype(np.int64)
    emb = np.asarray(inputs["emb"], dtype=np.float32)
    embp = np.zeros((V, EP), dtype=np.float32)
    embp[:, :E] = emb

    xg = x[g * BL : (g + 1) * BL]
    e = embp[xg]
    eT_f = np.ascontiguousarray(e.transpose(2, 1, 0).reshape(EP, NTOK))
    er = e[:, ::-1, :]
    eT_r = np.ascontiguousarray(er.transpose(2, 1, 0).reshape(EP, NTOK))

    def ktile(wT, kt):
        Kd, Gd = wT.shape
        assert Kd == kt * 128
        return np.ascontiguousarray(
            wT.reshape(kt, 128, Gd).transpose(1, 0, 2)
        ).astype(f16)

    def e3(eT):
        return np.ascontiguousarray(
            eT.reshape(3, 128, NTOK).transpose(1, 0, 2)
        ).astype(f16)

    def biascols(b_ih, b_hh):
        bv = b_ih.copy()
        bv[: 2 * H] += b_hh[: 2 * H]
        rows = bv.reshape(1, 12, 128).astype(f16)
        nbc = np.repeat(
            np.ascontiguousarray(b_hh[2 * H :].reshape(4, 128).T)[:, :, None],
            16, axis=2,
        ).astype(f16)
        return rows, nbc

    w_ih0 = np.asarray(inputs["w_ih0"], dtype=np.float32)
    w_hh0 = np.asarray(inputs["w_hh0"], dtype=np.float32)
    b_ih0 = np.asarray(inputs["b_ih0"], dtype=np.float32)
    b_hh0 = np.asarray(inputs["b_hh0"], dtype=np.float32)
    w_ih1 = np.asarray(inputs["w_ih1"], dtype=np.float32)
    w_hh1 = np.asarray(inputs["w_hh1"], dtype=np.float32)
    b_ih1 = np.asarray(inputs["b_ih1"], dtype=np.float32)
    b_hh1 = np.asarray(inputs["b_hh1"], dtype=np.float32)
    w1 = np.asarray(inputs["w1"], dtype=np.float32)

    d1, d2 = (0, 1) if role == 0 else (1, 0)
    dL = role
    own_half = slice(0, H) if role == 0 else slice(H, 2 * H)
    oth_half = slice(H, 2 * H) if role == 0 else slice(0, H)

    def wihT(d):
        w = np.zeros((G, EP), dtype=np.float32)
        w[:, :E] = w_ih0[d]
        return ktile(w.T, 3)

    b1c, n1c = biascols(b_ih0[d1], b_hh0[d1])
    b2c, n2c = biascols(b_ih0[d2], b_hh0[d2])
    bLc, nLc = biascols(b_ih1[dL], b_hh1[dL])

    m = {
        "eT1": e3(eT_f if role == 0 else eT_r),
        "eT2": e3(eT_r if role == 0 else eT_f),
        "wih1": wihT(d1),
        "wih2": wihT(d2),
        "whh1": ktile(w_hh0[d1].T, 4),
        "whh2": ktile(w_hh0[d2].T, 4),
        "wa": ktile(w_ih1[dL][:, own_half].T, 4),
        "wb": ktile(w_ih1[dL][:, oth_half].T, 4),
        "whhL": ktile(w_hh1[dL].T, 4),
        "biasr1": b1c, "biasr2": b2c, "biasrL": bLc,
        "ones": np.ones((1, 256), dtype=f16),
        "nbc1": n1c, "nbc2": n2c, "nbcL": nLc,
        "ident": np.eye(128, dtype=f16),
        "w1h": ktile(w1[:, own_half].T, 4),
    }
    return m


def kernel(**inputs) -> np.ndarray:
    from concourse.bass_utils import run_bass_kernel_spmd

    if "nc" not in _CACHE:
        _CACHE["nc"] = _build_nc()
    nc = _CACHE["nc"]

    core_ids = list(range(8))
    in_maps = []
    for c in core_ids:
        g, role = c % 4, c // 4
        in_maps.append(_prep_core_inputs(inputs, g, role))

    res = run_bass_kernel_spmd(nc, in_maps, core_ids)
    _CACHE["last_res"] = res

    b1 = np.asarray(inputs["b1"], dtype=np.float32)
    w2 = np.asarray(inputs["w2"], dtype=np.float32)
    b2 = np.asarray(inputs["b2"], dtype=np.float32)
    out = np.zeros((B, 2), dtype=np.float32)
    for g in range(4):
        p = (
            res.results[g]["headout"].astype(np.float32)
            + res.results[g + 4]["headout"].astype(np.float32)
        )
        hid = np.maximum(p + b1[:, None], 0.0)
        logits = w2 @ hid + b2[:, None]
        out[g * BL : (g + 1) * BL] = logits.T
    return out


# revision 34
# speedup vs baseline: 2.2549x; 1.0268x over previous
"""Bass/Trainium2 kernel for nn_GRUClassifier: 2-layer BiGRU + max-pool + MLP head.

Sharding: 8 cores = 4 batch groups x 2 L1-direction roles (as baseline).
Each core computes BOTH L0 directions for its 16 sequences, then one L1
direction, max-pool, and the W1 partial; host sums role-pair partials and
applies relu + W2.

Perf design (cost-model driven; the recurrence is dependency-chain-bound):
- The two L0 recurrences run INTERLEAVED step-by-step (independent chains)
  so engine latency of one chain hides under the other.
- Per-step op count: 5 DVE + 2 Act:
  * xp and the n-gate b_hh bias are injected into PSUM with identity
    matmuls (PE is nearly free), removing the gate-sum DVE ops.
    Accumulation groups are kept CONTIGUOUS per PSUM region (interleaved
    start-groups within a bank corrupt accumulation).
  * r,z sigmoids fused into one activation over [128,8,16]; separate PSUM
    tiles for rz vs n gates so the sigmoid does not wait on n matmuls
    (Tile tracks deps at whole-tile granularity).
  * h update: h = z*h + (1-z)*n with q=z*h computed off-chain and
    p=(z-1)*(-n) from the negated tanh output (z kept f32 to avoid
    cancellation in 1-z).
- xp blocks live entirely in SBUF ring tiles: the input-projection GEMMs
  write them directly (no DRAM roundtrip, no DMA races), sliced into small
  items paced between recurrence steps.
- y (L0 outputs) stay SBUF-resident; the backward chain writes y in
  position-reversed slots so BOTH halves of the L1 input GEMM accumulate
  in one PSUM pass (single xpL stream; no per-step peer adds).
"""
import os
import sys
import numpy as np

sys.path.insert(0, "/opt/trn_rl_repo")

B, T, E, H, V = 64, 256, 300, 512, 50000
EP = 384            # E padded to 3*128
G = 3 * H           # 1536 gate rows = 12 chunks of 128
BL = 16             # batch per core
NTOK = T * BL       # 4096
NBLK = 16           # 256-col xp blocks (16 steps each)

_CACHE = {}


def _patch_drain():
    """walrus CoreV3 rejects CTRL (Drain) instructions with too many sem
    waits; split the tail-drain's waits across preceding sync nops."""
    from concourse import mybir
    from concourse.tile import TileContext
    from concourse.vector_clock import ScopedClock

    if getattr(TileContext, "_drain_patched", False):
        return
    MAXW = 1

    def _drain_and_barrier(self, tick_clock, wait_clock):
        drain_inst = self.nc.sync.drain()
        wait_clock.add_sem_waits(
            drain_inst.ins, ScopedClock({None: tick_clock.global_clock})
        )
        si = drain_inst.ins.sync_info
        if si is not None and si.on_wait and len(si.on_wait) > MAXW:
            waits = list(si.on_wait)
            si.on_wait = waits[:MAXW]
            for i in range(MAXW, len(waits), MAXW):
                nop = self.nc.sync.nop(nofuse=True, hint="drain_wait_split")
                nsi = nop.ins.sync_info
                if nsi is None:
                    nop.ins.sync_info = mybir.SyncInfo(
                        on_wait=waits[i : i + MAXW], on_update=[]
                    )
                else:
                    nsi.on_wait = waits[i : i + MAXW]
        self.nc.all_engine_barrier()
        assert self.sems is not None
        popped = self.nc._tile_sem_poison_stack.pop()
        assert popped is self._sem_poison
        self.nc.clear_and_free_semaphores(list(self.sems.allocated().values()))
        self.nc.all_engine_barrier()

    TileContext._drain_and_barrier = _drain_and_barrier
    TileContext._drain_patched = True


def _split_multiwaits(nc, mybir, maxw=1):
    """walrus CoreV2/V3 setupSyncWait rejects instructions with more than one
    sem wait; split extras onto preceding same-engine nops."""
    cnt = 0
    for fn in nc.m.functions:
        for bb in fn.blocks:
            insts = bb.instructions
            out = []
            changed = False
            for inst in insts:
                si = getattr(inst, "sync_info", None)
                eng = getattr(inst, "engine", None)
                if (
                    si is not None
                    and si.on_wait
                    and len(si.on_wait) > maxw
                    and eng is not None
                    and eng != mybir.EngineType.Unassigned
                ):
                    waits = list(si.on_wait)
                    for w in waits[:-maxw]:
                        nop = mybir.InstNoOp(
                            name=f"ws_nop_{cnt}", ins=[], outs=[]
                        )
                        cnt += 1
                        nop.engine = eng
                        nop.sync_info = mybir.SyncInfo(
                            on_wait=[w], on_update=[]
                        )
                        out.append(nop)
                    si.on_wait = waits[-maxw:]
                    changed = True
                out.append(inst)
            if changed:
                bb.instructions = out


def _build_nc():
    from concourse import bass, mybir
    from concourse.tile import TileContext

    _patch_drain()
    f16 = mybir.dt.float16
    f32 = mybir.dt.float32
    AF = mybir.ActivationFunctionType
    OP = mybir.AluOpType

    nc = bass.Bass(target_bir_lowering=False)

    def par(name, shape, dt=f16, out=False):
        return nc.declare_dram_parameter(name, list(shape), dt, isOutput=out)

    eT1 = par("eT1", [128, 3, NTOK])
    eT2 = par("eT2", [128, 3, NTOK])
    wih1 = par("wih1", [128, 3, G])
    wih2 = par("wih2", [128, 3, G])
    whh1 = par("whh1", [128, 4, G])
    whh2 = par("whh2", [128, 4, G])
    wa = par("wa", [128, 4, G])
    wb = par("wb", [128, 4, G])
    whhL = par("whhL", [128, 4, G])
    biasr1 = par("biasr1", [1, 12, 128])
    biasr2 = par("biasr2", [1, 12, 128])
    biasrL = par("biasrL", [1, 12, 128])
    ones = par("ones", [1, 256])
    nbc1 = par("nbc1", [128, 4, 16])
    nbc2 = par("nbc2", [128, 4, 16])
    nbcL = par("nbcL", [128, 4, 16])
    ident = par("ident", [128, 128])
    w1h = par("w1h", [128, 4, 128])
    headout = par("headout", [128, 16], f32, out=True)
    DBG = os.environ.get("GRU_DEBUG_DUMP") == "1"
    if DBG:
        y1o = par("y1o", [128, 4, NTOK], out=True)
        y2o = par("y2o", [128, 4, NTOK], out=True)
        pooledo = par("pooledo", [128, 4, 16], out=True)

    with TileContext(nc) as tc:
        with (
            tc.tile_pool(name="wpool", bufs=1) as wp,
            tc.tile_pool(name="io", bufs=3) as io,
            tc.tile_pool(name="xpb", bufs=6) as xpb,
            tc.tile_pool(name="ew", bufs=2) as ew,
            tc.tile_pool(name="hp", bufs=2) as hp,
            tc.tile_pool(name="ps", bufs=2, space="PSUM") as ps,
            tc.tile_pool(name="psg", bufs=3, space="PSUM") as psg,
        ):
            def load(p, shape, dt=f16):
                t = wp.tile(list(shape), dt, tag=p.name + "_sb")
                nc.sync.dma_start(out=t[:], in_=p[:])
                return t

            # phase-A-critical weights first (SP DMA queue is in-order)
            wih1_s = load(wih1, [128, 3, G])
            biasr1_s = load(biasr1, [1, 12, 128])
            wih2_s = load(wih2, [128, 3, G])
            biasr2_s = load(biasr2, [1, 12, 128])
            ones_s = load(ones, [1, 256])
            ident_s = load(ident, [128, 128])
            nbc1_s = load(nbc1, [128, 4, 16])
            nbc2_s = load(nbc2, [128, 4, 16])
            whh1_s = load(whh1, [128, 4, G])
            whh2_s = load(whh2, [128, 4, G])

            def load_late(p, shape, dt=f16):
                # phase-B weights ride the idle GPSIMD DGE queue
                t = wp.tile(list(shape), dt, tag=p.name + "_sb")
                nc.gpsimd.dma_start(out=t[:], in_=p[:])
                return t

            wa_s = load_late(wa, [128, 4, G])
            wb_s = load_late(wb, [128, 4, G])
            whhL_s = load_late(whhL, [128, 4, G])
            biasrL_s = load_late(biasrL, [1, 12, 128])
            nbcL_s = load_late(nbcL, [128, 4, 16])
            w1h_s = load_late(w1h, [128, 4, 128])

            y1s = wp.tile([128, 4, NTOK], f16, tag="y1s")
            y2s = wp.tile([128, 4, NTOK], f16, tag="y2s")
            h0 = wp.tile([128, 4, 16], f16, tag="h0")
            nc.vector.memset(h0[:], 0.0)
            pooled = wp.tile([128, 4, 16], f16, tag="pooled")
            nc.vector.memset(pooled[:], -60000.0)

            ring = {}

            # ---------- GEMM item generators (finely sliced side work) ----
            # Each 256-col xp block is computed straight into an SBUF ring
            # tile; items are individually small so they pace between steps.
            # Bias is folded into the GEMM via a K=1 bias-row matmul, so the
            # PSUM->SBUF downcast is a bias-free Copy fused over 2 m-chunks
            # (fewer, Act-only fin ops keep the DVE queue clear for chains).
            def gemm_items_A(cid, eT_dram, w_sb, biasr_sb, blk):
                sl = slice(blk * 256, (blk + 1) * 256)
                st = {}
                items = []

                def open_():
                    ring[(cid, blk)] = xpb.tile(
                        [128, 12, 256], f16, tag="xpb", name="xpblk")
                    et = io.tile([128, 3, 256], f16, tag="et_in")
                    nc.sync.dma_start(out=et[:], in_=eT_dram[:, :, sl])
                    st["et"] = et

                items.append(open_)
                for m2 in range(6):
                    def mk(m2):
                        def mmsa():
                            p = ps.tile([128, 2, 256], f32, tag="gps")
                            st[m2] = p
                            m = 2 * m2
                            for k in range(3):
                                nc.tensor.matmul(
                                    p[:, 0, :],
                                    w_sb[:, k, m * 128 : (m + 1) * 128],
                                    st["et"][:, k, :],
                                    start=(k == 0), stop=False,
                                )
                            nc.tensor.matmul(
                                p[:, 0, :], biasr_sb[0:1, m, :],
                                ones_s[0:1, :], start=False, stop=True,
                            )

                        def mmsb():
                            p = st[m2]
                            m = 2 * m2 + 1
                            for k in range(3):
                                nc.tensor.matmul(
                                    p[:, 1, :],
                                    w_sb[:, k, m * 128 : (m + 1) * 128],
                                    st["et"][:, k, :],
                                    start=(k == 0), stop=False,
                                )
                            nc.tensor.matmul(
                                p[:, 1, :], biasr_sb[0:1, m, :],
                                ones_s[0:1, :], start=False, stop=True,
                            )

                        def fin():
                            p = st.pop(m2)
                            xb = ring[(cid, blk)]
                            nc.scalar.activation(
                                xb[:, 2 * m2 : 2 * m2 + 2, :], p[:], AF.Copy,
                            )

                        return mmsa, mmsb, fin

                    items += list(mk(m2))
                return items

            def gemm_items_L(blk):
                sl = slice(blk * 256, (blk + 1) * 256)
                st = {}
                items = []

                def open_():
                    ring[("L", blk)] = xpb.tile(
                        [128, 12, 256], f16, tag="xpb", name="xpblk")

                items.append(open_)
                for m2 in range(6):
                    def mk(m2):
                        def half(m, pi, p):
                            for k in range(4):
                                nc.tensor.matmul(
                                    p[:, pi, :],
                                    wa_s[:, k, m * 128 : (m + 1) * 128],
                                    y1s[:, k, sl],
                                    start=(k == 0), stop=False,
                                )
                            for k in range(4):
                                nc.tensor.matmul(
                                    p[:, pi, :],
                                    wb_s[:, k, m * 128 : (m + 1) * 128],
                                    y2s[:, k, sl],
                                    start=False, stop=False,
                                )
                            nc.tensor.matmul(
                                p[:, pi, :], biasrL_s[0:1, m, :],
                                ones_s[0:1, :], start=False, stop=True,
                            )

                        def mmsa():
                            p = ps.tile([128, 2, 256], f32, tag="gps")
                            st[m2] = p
                            half(2 * m2, 0, p)

                        def mmsb():
                            half(2 * m2 + 1, 1, st[m2])

                        def fin():
                            p = st.pop(m2)
                            xb = ring[("L", blk)]
                            nc.scalar.activation(
                                xb[:, 2 * m2 : 2 * m2 + 2, :], p[:], AF.Copy,
                            )

                        return mmsa, mmsb, fin

                    items += list(mk(m2))
                return items

            # ---------- recurrence chain -----------------------------------
            class Chain:
                def __init__(self, cid, whh_sb, nbc_sb,
                             y=None, rev=False, use_pool=False,
                             split_sigma=False, qp_split=False):
                    self.cid = cid
                    self.whh = whh_sb
                    self.nbc = nbc_sb
                    self.y = y
                    self.rev = rev
                    self.use_pool = use_pool
                    self.split_sigma = split_sigma
                    self.qp_split = qp_split
                    self.cur = None
                    self.hprev = h0
                    self.qprev = None
                    self.pprev = None

                def _col(self, t):
                    c = (T - 1 - t) if self.rev else t
                    return slice(c * 16, (c + 1) * 16)

                def step(self, t):
                    v = t % 16
                    if v == 0:
                        self.cur = ring.pop((self.cid, t // 16))
                    cur = self.cur
                    vs = slice(v * 16, (v + 1) * 16)
                    pn = psg.tile([128, 4, 16], f32, tag="pstn")
                    hp_ = self.hprev

                    def hrhs(k):
                        return (
                            hp_[:, k, :] if not isinstance(hp_, tuple)
                            else hp_[0][:, k, hp_[1]]
                        )

                    # One ungated injection matmul per PSUM tile opens that
                    # bank's single accumulation group (it runs ahead of the
                    # h-gated whh matmuls on the in-order PE); all whh
                    # matmuls then accumulate region-wise inside the open
                    # group, closing it on the last one.
                    def inj(dst, src):
                        nc.tensor.matmul(dst, ident_s[:], src,
                                         start=True, stop=False)

                    def whh_mms(dst, m0, m1, rhs=None, stop=True):
                        for m in range(m0, m1):
                            for k in range(4):
                                nc.tensor.matmul(
                                    dst[:, m - m0, :],
                                    self.whh[:, k, m * 128 : (m + 1) * 128],
                                    hrhs(k) if rhs is None else rhs[:, k, :],
                                    start=False,
                                    stop=(stop and m == m1 - 1 and k == 3),
                                )

                    if self.qp_split:
                        # gh = Whh*q + Whh*p (exact); the q half fires as
                        # soon as q(t-1) exists (mid-chain), only the p half
                        # gates the sigmoid -> h drops off the critical path.
                        pr = psg.tile([128, 4, 16], f32, tag="pstrz",
                                      name="pr")
                        pz = psg.tile([128, 4, 16], f32, tag="pstrz",
                                      name="pz")
                        if t == 0:
                            nc.tensor.matmul(pn[:], ident_s[:], self.nbc[:],
                                             start=True, stop=True)
                            nc.tensor.matmul(pr[:], ident_s[:],
                                             cur[:, 0:4, vs],
                                             start=True, stop=True)
                            nc.tensor.matmul(pz[:], ident_s[:],
                                             cur[:, 4:8, vs],
                                             start=True, stop=True)
                        else:
                            inj(pn[:], self.nbc[:])
                            inj(pr[:], cur[:, 0:4, vs])
                            inj(pz[:], cur[:, 4:8, vs])
                            qq, pp = self.qprev, self.pprev
                            whh_mms(pr, 0, 4, rhs=qq, stop=False)
                            whh_mms(pz, 4, 8, rhs=qq, stop=False)
                            whh_mms(pn, 8, 12, rhs=qq, stop=False)
                            whh_mms(pr, 0, 4, rhs=pp)
                            whh_mms(pz, 4, 8, rhs=pp)
                            whh_mms(pn, 8, 12, rhs=pp)
                        rt = ew.tile([128, 4, 16], f32, tag=f"rt{self.cid}")
                        zt = ew.tile([128, 4, 16], f32, tag=f"zt{self.cid}")
                        nc.scalar.activation(rt[:], pr[:], AF.Sigmoid)
                        nc.scalar.activation(zt[:], pz[:], AF.Sigmoid)
                        rr, zz = rt[:], zt[:]
                    elif self.split_sigma:
                        inj(pn[:], self.nbc[:])
                        pr = psg.tile([128, 4, 16], f32, tag="pstrz",
                                      name="pr")
                        pz = psg.tile([128, 4, 16], f32, tag="pstrz",
                                      name="pz")
                        inj(pr[:], cur[:, 0:4, vs])
                        inj(pz[:], cur[:, 4:8, vs])
                        whh_mms(pr, 0, 4)
                        whh_mms(pz, 4, 8)
                        whh_mms(pn, 8, 12)
                        rt = ew.tile([128, 4, 16], f32, tag=f"rt{self.cid}")
                        zt = ew.tile([128, 4, 16], f32, tag=f"zt{self.cid}")
                        nc.scalar.activation(rt[:], pr[:], AF.Sigmoid)
                        nc.scalar.activation(zt[:], pz[:], AF.Sigmoid)
                        rr, zz = rt[:], zt[:]
                    else:
                        inj(pn[:], self.nbc[:])
                        prz = psg.tile([128, 8, 16], f32, tag="pstrz",
                                       name="prz")
                        inj(prz[:], cur[:, 0:8, vs])
                        whh_mms(prz, 0, 8)
                        whh_mms(pn, 8, 12)
                        rz = ew.tile([128, 8, 16], f32, tag=f"rz{self.cid}")
                        nc.scalar.activation(rz[:], prz[:], AF.Sigmoid)
                        rr, zz = rz[:, 0:4, :], rz[:, 4:8, :]
                    u = ew.tile([128, 4, 16], f32, tag=f"u{self.cid}")
                    nc.vector.scalar_tensor_tensor(
                        out=u[:], in0=pn[:], scalar=1.0,
                        in1=rr, op0=OP.mult, op1=OP.mult,
                    )
                    tn = ew.tile([128, 4, 16], f32, tag=f"tn{self.cid}")
                    nc.vector.scalar_tensor_tensor(
                        out=tn[:], in0=u[:], scalar=1.0,
                        in1=cur[:, 8:12, vs], op0=OP.mult, op1=OP.add,
                    )
                    nn = ew.tile([128, 4, 16], f16, tag=f"nn{self.cid}")
                    nc.scalar.activation(nn[:], tn[:], AF.Tanh, scale=-1.0)
                    q = ew.tile([128, 4, 16], f16, tag=f"q{self.cid}")
                    nc.vector.scalar_tensor_tensor(
                        out=q[:], in0=zz, scalar=1.0,
                        in1=hp_[:] if not isinstance(hp_, tuple)
                        else hp_[0][:, :, hp_[1]],
                        op0=OP.mult, op1=OP.mult,
                    )
                    p = ew.tile([128, 4, 16], f16, tag=f"p{self.cid}")
                    nc.vector.scalar_tensor_tensor(
                        out=p[:], in0=zz, scalar=1.0,
                        in1=nn[:], op0=OP.subtract, op1=OP.mult,
                    )
                    if self.y is not None:
                        cs = self._col(t)
                        nc.vector.scalar_tensor_tensor(
                            out=self.y[:, :, cs], in0=q[:], scalar=1.0,
                            in1=p[:], op0=OP.mult, op1=OP.add,
                        )
                        self.hprev = (self.y, cs)
                    else:
                        hn = hp.tile([128, 4, 16], f16, tag="hL")
                        nc.vector.scalar_tensor_tensor(
                            out=hn[:], in0=q[:], scalar=1.0,
                            in1=p[:], op0=OP.mult, op1=OP.add,
                        )
                        if self.use_pool:
                            nc.vector.scalar_tensor_tensor(
                                out=pooled[:], in0=hn[:], scalar=1.0,
                                in1=pooled[:], op0=OP.mult, op1=OP.max,
                            )
                        self.hprev = hn
                    self.qprev = q
                    self.pprev = p

            # ---------- item pacing ---------------------------------------
            def paced(items, t0, t1):
                n = len(items)
                w = max(1, t1 - t0)
                return [(t0 + (i * w) // n, fn) for i, fn in enumerate(items)]

            def run_phase(chains, sched):
                sched = sorted(sched, key=lambda x: x[0])
                si = 0
                for t in range(T):
                    while si < len(sched) and sched[si][0] <= t:
                        sched[si][1]()
                        si += 1
                    for c in chains:
                        c.step(t)
                while si < len(sched):
                    sched[si][1]()
                    si += 1

            # ---------- phase A: L0 fwd + bwd interleaved -----------------
            for fn in gemm_items_A("A", eT1, wih1_s, biasr1_s, 0):
                fn()
            for fn in gemm_items_A("B", eT2, wih2_s, biasr2_s, 0):
                fn()
            chA = Chain("A", whh1_s, nbc1_s, y=y1s, rev=False)
            chB = Chain("B", whh2_s, nbc2_s, y=y2s, rev=True)
            schedA = []
            for c in range(1, NBLK):
                schedA += paced(
                    gemm_items_A("A", eT1, wih1_s, biasr1_s, c)
                    + gemm_items_A("B", eT2, wih2_s, biasr2_s, c),
                    max(0, 16 * c - 44), 16 * c - 8,
                )
            run_phase([chA, chB], schedA)

            # ---------- phase B: L1 (merged xpL GEMM) ---------------------
            for fn in gemm_items_L(0):
                fn()
            chL = Chain("L", whhL_s, nbcL_s, use_pool=True,
                        split_sigma=True, qp_split=True)
            schedB = []
            for c in range(1, NBLK):
                schedB += paced(
                    gemm_items_L(c), max(0, 16 * c - 44), 16 * c - 8
                )
            run_phase([chL], schedB)

            # ---------- head ----------------------------------------------
            hd = psg.tile([128, 4, 16], f32, tag="pstn")
            for k in range(4):
                nc.tensor.matmul(
                    hd[:, 0, :], w1h_s[:, k, :], pooled[:, k, :],
                    start=(k == 0), stop=(k == 3),
                )
            ho = io.tile([128, 16], f32, tag="ho")
            nc.vector.tensor_copy(out=ho[:], in_=hd[:, 0, :])
            nc.sync.dma_start(out=headout[:], in_=ho[:])
            if DBG:
                nc.sync.dma_start(out=y1o[:], in_=y1s[:])
                nc.sync.dma_start(out=y2o[:], in_=y2s[:])
                nc.sync.dma_start(out=pooledo[:], in_=pooled[:])

    _split_multiwaits(nc, mybir)
    try:
        ents = getattr(tc, "_perfetto_entries", None)
        span = None
        if ents:
            starts = [e[1] for e in ents if e[1] is not None]
            ends = [e[2] if e[2] is not None else e[1] for e in ents]
            if starts and ends:
                span = int(max(ends) - min(starts))
        _CACHE["model_ns"] = span
    except Exception:
        _CACHE["model_ns"] = None
    return nc


def _prep_core_inputs(inputs, g, role):
    """Host-side sharding/layout prep for core (batch group g, role)."""
    f16 = np.float16
    x = np.asarray(inputs["x"]).astype(np.int64)
    emb = np.asarray(inputs["emb"], dtype=np.float32)
    embp = np.zeros((V, EP), dtype=np.float32)
    embp[:, :E] = emb

    xg = x[g * BL : (g + 1) * BL]
    e = embp[xg]
    eT_f = np.ascontiguousarray(e.transpose(2, 1, 0).reshape(EP, NTOK))
    er = e[:, ::-1, :]
    eT_r = np.ascontiguousarray(er.transpose(2, 1, 0).reshape(EP, NTOK))

    def ktile(wT, kt):
        Kd, Gd = wT.shape
        assert Kd == kt * 128
        return np.ascontiguousarray(
            wT.reshape(kt, 128, Gd).transpose(1, 0, 2)
        ).astype(f16)

    def e3(eT):
        return np.ascontiguousarray(
            eT.reshape(3, 128, NTOK).transpose(1, 0, 2)
        ).astype(f16)

    def biascols(b_ih, b_hh):
        bv = b_ih.copy()
        bv[: 2 * H] += b_hh[: 2 * H]
        rows = bv.reshape(1, 12, 128).astype(f16)
        nbc = np.repeat(
            np.ascontiguousarray(b_hh[2 * H :].reshape(4, 128).T)[:, :, None],
            16, axis=2,
        ).astype(f16)
        return rows, nbc

    w_ih0 = np.asarray(inputs["w_ih0"], dtype=np.float32)
    w_hh0 = np.asarray(inputs["w_hh0"], dtype=np.float32)
    b_ih0 = np.asarray(inputs["b_ih0"], dtype=np.float32)
    b_hh0 = np.asarray(inputs["b_hh0"], dtype=np.float32)
    w_ih1 = np.asarray(inputs["w_ih1"], dtype=np.float32)
    w_hh1 = np.asarray(inputs["w_hh1"], dtype=np.float32)
    b_ih1 = np.asarray(inputs["b_ih1"], dtype=np.float32)
    b_hh1 = np.asarray(inputs["b_hh1"], dtype=np.float32)
    w1 = np.asarray(inputs["w1"], dtype=np.float32)

    d1, d2 = (0, 1) if role == 0 else (1, 0)
    dL = role
    own_half = slice(0, H) if role == 0 else slice(H, 2 * H)
    oth_half = slice(H, 2 * H) if role == 0 else slice(0, H)

    def wihT(d):
        w = np.zeros((G, EP), dtype=np.float32)
        w[:, :E] = w_ih0[d]
        return ktile(w.T, 3)

    b1c, n1c = biascols(b_ih0[d1], b_hh0[d1])
    b2c, n2c = biascols(b_ih0[d2], b_hh0[d2])
    bLc, nLc = biascols(b_ih1[dL], b_hh1[dL])

    m = {
        "eT1": e3(eT_f if role == 0 else eT_r),
        "eT2": e3(eT_r if role == 0 else eT_f),
        "wih1": wihT(d1),
        "wih2": wihT(d2),
        "whh1": ktile(w_hh0[d1].T, 4),
        "whh2": ktile(w_hh0[d2].T, 4),
        "wa": ktile(w_ih1[dL][:, own_half].T, 4),
        "wb": ktile(w_ih1[dL][:, oth_half].T, 4),
        "whhL": ktile(w_hh1[dL].T, 4),
        "biasr1": b1c, "biasr2": b2c, "biasrL": bLc,
        "ones": np.ones((1, 256), dtype=f16),
        "nbc1": n1c, "nbc2": n2c, "nbcL": nLc,
        "ident": np.eye(128, dtype=f16),
        "w1h": ktile(w1[:, own_half].T, 4),
    }
    return m


def kernel(**inputs) -> np.ndarray:
    from concourse.bass_utils import run_bass_kernel_spmd

    if "nc" not in _CACHE:
        _CACHE["nc"] = _build_nc()
    nc = _CACHE["nc"]

    core_ids = list(range(8))
    in_maps = []
    for c in core_ids:
        g, role = c % 4, c // 4
        in_maps.append(_prep_core_inputs(inputs, g, role))

    res = run_bass_kernel_spmd(nc, in_maps, core_ids)
    _CACHE["last_res"] = res

    b1 = np.asarray(inputs["b1"], dtype=np.float32)
    w2 = np.asarray(inputs["w2"], dtype=np.float32)
    b2 = np.asarray(inputs["b2"], dtype=np.float32)
    out = np.zeros((B, 2), dtype=np.float32)
    for g in range(4):
        p = (
            res.results[g]["headout"].astype(np.float32)
            + res.results[g + 4]["headout"].astype(np.float32)
        )
        hid = np.maximum(p + b1[:, None], 0.0)
        logits = w2 @ hid + b2[:, None]
        out[g * BL : (g + 1) * BL] = logits.T
    return out


# revision 38
# speedup vs baseline: 2.3991x; 1.0639x over previous
"""Bass/Trainium2 kernel for nn_GRUClassifier: 2-layer BiGRU + max-pool + MLP head.

Sharding: 8 cores = 4 batch groups x 2 L1-direction roles (as baseline).
Each core computes BOTH L0 directions for its 16 sequences, then one L1
direction, max-pool, and the W1 partial; host sums role-pair partials and
applies relu + W2.

Perf design (cost-model driven; the recurrence is dependency-chain-bound):
- The two L0 recurrences run INTERLEAVED step-by-step (independent chains)
  so engine latency of one chain hides under the other.
- Per-step op count: 5 DVE + 2 Act:
  * xp and the n-gate b_hh bias are injected into PSUM with identity
    matmuls (PE is nearly free), removing the gate-sum DVE ops.
    Accumulation groups are kept CONTIGUOUS per PSUM region (interleaved
    start-groups within a bank corrupt accumulation).
  * r,z sigmoids fused into one activation over [128,8,16]; separate PSUM
    tiles for rz vs n gates so the sigmoid does not wait on n matmuls
    (Tile tracks deps at whole-tile granularity).
  * h update: h = z*h + (1-z)*n with q=z*h computed off-chain and
    p=(z-1)*(-n) from the negated tanh output (z kept f32 to avoid
    cancellation in 1-z).
- xp blocks live entirely in SBUF ring tiles: the input-projection GEMMs
  write them directly (no DRAM roundtrip, no DMA races), sliced into small
  items paced between recurrence steps.
- y (L0 outputs) stay SBUF-resident; the backward chain writes y in
  position-reversed slots so BOTH halves of the L1 input GEMM accumulate
  in one PSUM pass (single xpL stream; no per-step peer adds).
"""
import os
import sys
import numpy as np

sys.path.insert(0, "/opt/trn_rl_repo")

B, T, E, H, V = 64, 256, 300, 512, 50000
EP = 384            # E padded to 3*128
G = 3 * H           # 1536 gate rows = 12 chunks of 128
BL = 16             # batch per core
NTOK = T * BL       # 4096
NBLK = 16           # 256-col xp blocks (16 steps each)

_CACHE = {}


def _patch_drain():
    """walrus CoreV3 rejects CTRL (Drain) instructions with too many sem
    waits; split the tail-drain's waits across preceding sync nops."""
    from concourse import mybir
    from concourse.tile import TileContext
    from concourse.vector_clock import ScopedClock

    if getattr(TileContext, "_drain_patched", False):
        return
    MAXW = 1

    def _drain_and_barrier(self, tick_clock, wait_clock):
        drain_inst = self.nc.sync.drain()
        wait_clock.add_sem_waits(
            drain_inst.ins, ScopedClock({None: tick_clock.global_clock})
        )
        si = drain_inst.ins.sync_info
        if si is not None and si.on_wait and len(si.on_wait) > MAXW:
            waits = list(si.on_wait)
            si.on_wait = waits[:MAXW]
            for i in range(MAXW, len(waits), MAXW):
                nop = self.nc.sync.nop(nofuse=True, hint="drain_wait_split")
                nsi = nop.ins.sync_info
                if nsi is None:
                    nop.ins.sync_info = mybir.SyncInfo(
                        on_wait=waits[i : i + MAXW], on_update=[]
                    )
                else:
                    nsi.on_wait = waits[i : i + MAXW]
        self.nc.all_engine_barrier()
        assert self.sems is not None
        popped = self.nc._tile_sem_poison_stack.pop()
        assert popped is self._sem_poison
        self.nc.clear_and_free_semaphores(list(self.sems.allocated().values()))
        self.nc.all_engine_barrier()

    TileContext._drain_and_barrier = _drain_and_barrier
    TileContext._drain_patched = True


def _split_multiwaits(nc, mybir, maxw=1):
    """walrus CoreV2/V3 setupSyncWait rejects instructions with more than one
    sem wait; split extras onto preceding same-engine nops."""
    cnt = 0
    for fn in nc.m.functions:
        for bb in fn.blocks:
            insts = bb.instructions
            out = []
            changed = False
            for inst in insts:
                si = getattr(inst, "sync_info", None)
                eng = getattr(inst, "engine", None)
                if (
                    si is not None
                    and si.on_wait
                    and len(si.on_wait) > maxw
                    and eng is not None
                    and eng != mybir.EngineType.Unassigned
                ):
                    waits = list(si.on_wait)
                    for w in waits[:-maxw]:
                        nop = mybir.InstNoOp(
                            name=f"ws_nop_{cnt}", ins=[], outs=[]
                        )
                        cnt += 1
                        nop.engine = eng
                        nop.sync_info = mybir.SyncInfo(
                            on_wait=[w], on_update=[]
                        )
                        out.append(nop)
                    si.on_wait = waits[-maxw:]
                    changed = True
                out.append(inst)
            if changed:
                bb.instructions = out


def _build_nc():
    from concourse import bass, mybir
    from concourse.tile import TileContext

    _patch_drain()
    f16 = mybir.dt.float16
    f32 = mybir.dt.float32
    AF = mybir.ActivationFunctionType
    OP = mybir.AluOpType

    nc = bass.Bass(target_bir_lowering=False)

    def par(name, shape, dt=f16, out=False):
        return nc.declare_dram_parameter(name, list(shape), dt, isOutput=out)

    eT1 = par("eT1", [128, 3, NTOK])
    eT2 = par("eT2", [128, 3, NTOK])
    wih1 = par("wih1", [128, 3, G])
    wih2 = par("wih2", [128, 3, G])
    whh1 = par("whh1", [128, 4, G])
    whh2 = par("whh2", [128, 4, G])
    wa = par("wa", [128, 4, G])
    wb = par("wb", [128, 4, G])
    whhL = par("whhL", [128, 4, G])
    biasr1 = par("biasr1", [1, 12, 128])
    biasr2 = par("biasr2", [1, 12, 128])
    biasrL = par("biasrL", [1, 12, 128])
    ones = par("ones", [1, 256])
    nbc1 = par("nbc1", [128, 4, 16])
    nbc2 = par("nbc2", [128, 4, 16])
    nbcL = par("nbcL", [128, 4, 16])
    ident = par("ident", [128, 128])
    w1h = par("w1h", [128, 4, 128])
    headout = par("headout", [128, 16], f32, out=True)
    DBG = os.environ.get("GRU_DEBUG_DUMP") == "1"
    if DBG:
        y1o = par("y1o", [128, 4, NTOK], out=True)
        y2o = par("y2o", [128, 4, NTOK], out=True)
        pooledo = par("pooledo", [128, 4, 16], out=True)

    with TileContext(nc) as tc:
        with (
            tc.tile_pool(name="wpool", bufs=1) as wp,
            tc.tile_pool(name="io", bufs=3) as io,
            tc.tile_pool(name="xpb", bufs=6) as xpb,
            tc.tile_pool(name="ew", bufs=2) as ew,
            tc.tile_pool(name="hp", bufs=2) as hp,
            tc.tile_pool(name="ps", bufs=2, space="PSUM") as ps,
            tc.tile_pool(name="psg", bufs=3, space="PSUM") as psg,
        ):
            def load(p, shape, dt=f16):
                t = wp.tile(list(shape), dt, tag=p.name + "_sb")
                nc.sync.dma_start(out=t[:], in_=p[:])
                return t

            # phase-A-critical weights first (SP DMA queue is in-order)
            wih1_s = load(wih1, [128, 3, G])
            biasr1_s = load(biasr1, [1, 12, 128])
            wih2_s = load(wih2, [128, 3, G])
            biasr2_s = load(biasr2, [1, 12, 128])
            ones_s = load(ones, [1, 256])
            ident_s = load(ident, [128, 128])
            nbc1_s = load(nbc1, [128, 4, 16])
            nbc2_s = load(nbc2, [128, 4, 16])

            def load_late(p, shape, dt=f16):
                # phase-B weights ride the idle GPSIMD DGE queue
                t = wp.tile(list(shape), dt, tag=p.name + "_sb")
                nc.gpsimd.dma_start(out=t[:], in_=p[:])
                return t

            wa_s = load_late(wa, [128, 4, G])
            wb_s = load_late(wb, [128, 4, G])
            whhL_s = load_late(whhL, [128, 4, G])
            biasrL_s = load_late(biasrL, [1, 12, 128])
            nbcL_s = load_late(nbcL, [128, 4, 16])
            w1h_s = load_late(w1h, [128, 4, 128])

            y1s = wp.tile([128, 4, NTOK], f16, tag="y1s")
            y2s = wp.tile([128, 4, NTOK], f16, tag="y2s")
            h0 = wp.tile([128, 4, 16], f16, tag="h0")
            nc.vector.memset(h0[:], 0.0)
            pooled = wp.tile([128, 4, 16], f16, tag="pooled")
            nc.vector.memset(pooled[:], -60000.0)

            ring = {}

            # ---------- GEMM item generators (finely sliced side work) ----
            # Each 256-col xp block is computed straight into an SBUF ring
            # tile; items are individually small so they pace between steps.
            # Bias is folded into the GEMM via a K=1 bias-row matmul, so the
            # PSUM->SBUF downcast is a bias-free Copy fused over 2 m-chunks
            # (fewer, Act-only fin ops keep the DVE queue clear for chains).
            def gemm_items_A(cid, eT_dram, w_sb, biasr_sb, blk):
                sl = slice(blk * 256, (blk + 1) * 256)
                st = {}
                items = []

                def open_():
                    ring[(cid, blk)] = xpb.tile(
                        [128, 12, 256], f16, tag="xpb", name="xpblk")
                    et = io.tile([128, 3, 256], f16, tag="et_in")
                    nc.sync.dma_start(out=et[:], in_=eT_dram[:, :, sl])
                    st["et"] = et

                items.append(open_)
                for m2 in range(6):
                    def mk(m2):
                        def mmsa():
                            p = ps.tile([128, 2, 256], f32, tag="gps")
                            st[m2] = p
                            m = 2 * m2
                            for k in range(3):
                                nc.tensor.matmul(
                                    p[:, 0, :],
                                    w_sb[:, k, m * 128 : (m + 1) * 128],
                                    st["et"][:, k, :],
                                    start=(k == 0), stop=False,
                                )
                            nc.tensor.matmul(
                                p[:, 0, :], biasr_sb[0:1, m, :],
                                ones_s[0:1, :], start=False, stop=True,
                            )

                        def mmsb():
                            p = st[m2]
                            m = 2 * m2 + 1
                            for k in range(3):
                                nc.tensor.matmul(
                                    p[:, 1, :],
                                    w_sb[:, k, m * 128 : (m + 1) * 128],
                                    st["et"][:, k, :],
                                    start=(k == 0), stop=False,
                                )
                            nc.tensor.matmul(
                                p[:, 1, :], biasr_sb[0:1, m, :],
                                ones_s[0:1, :], start=False, stop=True,
                            )

                        def fin():
                            p = st.pop(m2)
                            xb = ring[(cid, blk)]
                            nc.scalar.activation(
                                xb[:, 2 * m2 : 2 * m2 + 2, :], p[:], AF.Copy,
                            )

                        return mmsa, mmsb, fin

                    items += list(mk(m2))
                return items

            def gemm_items_L(blk):
                sl = slice(blk * 256, (blk + 1) * 256)
                st = {}
                items = []

                def open_():
                    ring[("L", blk)] = xpb.tile(
                        [128, 12, 256], f16, tag="xpb", name="xpblk")

                items.append(open_)
                for m2 in range(6):
                    def mk(m2):
                        def half(m, pi, p):
                            for k in range(4):
                                nc.tensor.matmul(
                                    p[:, pi, :],
                                    wa_s[:, k, m * 128 : (m + 1) * 128],
                                    y1s[:, k, sl],
                                    start=(k == 0), stop=False,
                                )
                            for k in range(4):
                                nc.tensor.matmul(
                                    p[:, pi, :],
                                    wb_s[:, k, m * 128 : (m + 1) * 128],
                                    y2s[:, k, sl],
                                    start=False, stop=False,
                                )
                            nc.tensor.matmul(
                                p[:, pi, :], biasrL_s[0:1, m, :],
                                ones_s[0:1, :], start=False, stop=True,
                            )

                        def mmsa():
                            p = ps.tile([128, 2, 256], f32, tag="gps")
                            st[m2] = p
                            half(2 * m2, 0, p)

                        def mmsb():
                            half(2 * m2 + 1, 1, st[m2])

                        def fin():
                            p = st.pop(m2)
                            xb = ring[("L", blk)]
                            nc.scalar.activation(
                                xb[:, 2 * m2 : 2 * m2 + 2, :], p[:], AF.Copy,
                            )

                        return mmsa, mmsb, fin

                    items += list(mk(m2))
                return items

            # ---------- recurrence chain -----------------------------------
            class Chain:
                def __init__(self, cid, whh_sb, nbc_sb,
                             y=None, rev=False, use_pool=False,
                             split_sigma=False, qp_split=False):
                    self.cid = cid
                    self.whh = whh_sb
                    self.nbc = nbc_sb
                    self.y = y
                    self.rev = rev
                    self.use_pool = use_pool
                    self.split_sigma = split_sigma
                    self.qp_split = qp_split
                    self.cur = None
                    self.hprev = h0
                    self.qprev = None
                    self.pprev = None

                def _col(self, t):
                    c = (T - 1 - t) if self.rev else t
                    return slice(c * 16, (c + 1) * 16)

                def step(self, t):
                    v = t % 16
                    if v == 0:
                        self.cur = ring.pop((self.cid, t // 16))
                    cur = self.cur
                    vs = slice(v * 16, (v + 1) * 16)
                    pn = psg.tile([128, 4, 16], f32, tag="pstn")
                    hp_ = self.hprev

                    def hrhs(k):
                        return (
                            hp_[:, k, :] if not isinstance(hp_, tuple)
                            else hp_[0][:, k, hp_[1]]
                        )

                    # One ungated injection matmul per PSUM tile opens that
                    # bank's single accumulation group (it runs ahead of the
                    # h-gated whh matmuls on the in-order PE); all whh
                    # matmuls then accumulate region-wise inside the open
                    # group, closing it on the last one.
                    def inj(dst, src):
                        nc.tensor.matmul(dst, ident_s[:], src,
                                         start=True, stop=False)

                    def whh_mms(dst, m0, m1, rhs=None, stop=True):
                        for m in range(m0, m1):
                            for k in range(4):
                                nc.tensor.matmul(
                                    dst[:, m - m0, :],
                                    self.whh[:, k, m * 128 : (m + 1) * 128],
                                    hrhs(k) if rhs is None else rhs[:, k, :],
                                    start=False,
                                    stop=(stop and m == m1 - 1 and k == 3),
                                )

                    if self.qp_split and self.split_sigma:
                        # gh = Whh*q + Whh*p (exact); the q half fires as
                        # soon as q(t-1) exists (mid-chain), only the p half
                        # gates the sigmoid -> h drops off the critical path.
                        pr = psg.tile([128, 4, 16], f32, tag="pstrz",
                                      name="pr")
                        pz = psg.tile([128, 4, 16], f32, tag="pstrz",
                                      name="pz")
                        if t == 0:
                            nc.tensor.matmul(pn[:], ident_s[:], self.nbc[:],
                                             start=True, stop=True)
                            nc.tensor.matmul(pr[:], ident_s[:],
                                             cur[:, 0:4, vs],
                                             start=True, stop=True)
                            nc.tensor.matmul(pz[:], ident_s[:],
                                             cur[:, 4:8, vs],
                                             start=True, stop=True)
                        else:
                            inj(pn[:], self.nbc[:])
                            inj(pr[:], cur[:, 0:4, vs])
                            inj(pz[:], cur[:, 4:8, vs])
                            qq, pp = self.qprev, self.pprev
                            whh_mms(pr, 0, 4, rhs=qq, stop=False)
                            whh_mms(pz, 4, 8, rhs=qq, stop=False)
                            whh_mms(pn, 8, 12)
                            whh_mms(pr, 0, 4, rhs=pp)
                            whh_mms(pz, 4, 8, rhs=pp)
                        rt = ew.tile([128, 4, 16], f32, tag=f"rt{self.cid}")
                        zt = ew.tile([128, 4, 16], f32, tag=f"zt{self.cid}")
                        nc.scalar.activation(rt[:], pr[:], AF.Sigmoid)
                        nc.scalar.activation(zt[:], pz[:], AF.Sigmoid)
                        rr, zz = rt[:], zt[:]
                    elif self.qp_split:
                        prz = psg.tile([128, 8, 16], f32, tag="pstrz",
                                       name="prz")
                        if t == 0:
                            nc.tensor.matmul(pn[:], ident_s[:], self.nbc[:],
                                             start=True, stop=True)
                            nc.tensor.matmul(prz[:], ident_s[:],
                                             cur[:, 0:8, vs],
                                             start=True, stop=True)
                        else:
                            inj(pn[:], self.nbc[:])
                            inj(prz[:], cur[:, 0:8, vs])
                            qq, pp = self.qprev, self.pprev
                            whh_mms(prz, 0, 8, rhs=qq, stop=False)
                            whh_mms(pn, 8, 12)
                            whh_mms(prz, 0, 8, rhs=pp)
                        rz = ew.tile([128, 8, 16], f32, tag=f"rz{self.cid}")
                        nc.scalar.activation(rz[:], prz[:], AF.Sigmoid)
                        rr, zz = rz[:, 0:4, :], rz[:, 4:8, :]
                    elif self.split_sigma:
                        inj(pn[:], self.nbc[:])
                        pr = psg.tile([128, 4, 16], f32, tag="pstrz",
                                      name="pr")
                        pz = psg.tile([128, 4, 16], f32, tag="pstrz",
                                      name="pz")
                        inj(pr[:], cur[:, 0:4, vs])
                        inj(pz[:], cur[:, 4:8, vs])
                        whh_mms(pr, 0, 4)
                        whh_mms(pz, 4, 8)
                        whh_mms(pn, 8, 12)
                        rt = ew.tile([128, 4, 16], f32, tag=f"rt{self.cid}")
                        zt = ew.tile([128, 4, 16], f32, tag=f"zt{self.cid}")
                        nc.scalar.activation(rt[:], pr[:], AF.Sigmoid)
                        nc.scalar.activation(zt[:], pz[:], AF.Sigmoid)
                        rr, zz = rt[:], zt[:]
                    else:
                        inj(pn[:], self.nbc[:])
                        prz = psg.tile([128, 8, 16], f32, tag="pstrz",
                                       name="prz")
                        inj(prz[:], cur[:, 0:8, vs])
                        whh_mms(prz, 0, 8)
                        whh_mms(pn, 8, 12)
                        rz = ew.tile([128, 8, 16], f32, tag=f"rz{self.cid}")
                        nc.scalar.activation(rz[:], prz[:], AF.Sigmoid)
                        rr, zz = rz[:, 0:4, :], rz[:, 4:8, :]
                    u = ew.tile([128, 4, 16], f32, tag=f"u{self.cid}")
                    nc.vector.scalar_tensor_tensor(
                        out=u[:], in0=pn[:], scalar=1.0,
                        in1=rr, op0=OP.mult, op1=OP.mult,
                    )
                    tn = ew.tile([128, 4, 16], f32, tag=f"tn{self.cid}")
                    nc.vector.scalar_tensor_tensor(
                        out=tn[:], in0=u[:], scalar=1.0,
                        in1=cur[:, 8:12, vs], op0=OP.mult, op1=OP.add,
                    )
                    nn = ew.tile([128, 4, 16], f16, tag=f"nn{self.cid}")
                    nc.scalar.activation(nn[:], tn[:], AF.Tanh, scale=-1.0)
                    q = ew.tile([128, 4, 16], f16, tag=f"q{self.cid}")
                    nc.vector.scalar_tensor_tensor(
                        out=q[:], in0=zz, scalar=1.0,
                        in1=hp_[:] if not isinstance(hp_, tuple)
                        else hp_[0][:, :, hp_[1]],
                        op0=OP.mult, op1=OP.mult,
                    )
                    p = ew.tile([128, 4, 16], f16, tag=f"p{self.cid}")
                    nc.vector.scalar_tensor_tensor(
                        out=p[:], in0=zz, scalar=1.0,
                        in1=nn[:], op0=OP.subtract, op1=OP.mult,
                    )
                    if self.y is not None:
                        cs = self._col(t)
                        nc.vector.scalar_tensor_tensor(
                            out=self.y[:, :, cs], in0=q[:], scalar=1.0,
                            in1=p[:], op0=OP.mult, op1=OP.add,
                        )
                        self.hprev = (self.y, cs)
                    else:
                        hn = hp.tile([128, 4, 16], f16, tag="hL")
                        nc.vector.scalar_tensor_tensor(
                            out=hn[:], in0=q[:], scalar=1.0,
                            in1=p[:], op0=OP.mult, op1=OP.add,
                        )
                        if self.use_pool:
                            nc.vector.scalar_tensor_tensor(
                                out=pooled[:], in0=hn[:], scalar=1.0,
                                in1=pooled[:], op0=OP.mult, op1=OP.max,
                            )
                        self.hprev = hn
                    self.qprev = q
                    self.pprev = p

            # ---------- item pacing ---------------------------------------
            def paced(items, t0, t1):
                n = len(items)
                w = max(1, t1 - t0)
                return [(t0 + (i * w) // n, fn) for i, fn in enumerate(items)]

            def run_phase(chains, sched):
                sched = sorted(sched, key=lambda x: x[0])
                si = 0
                for t in range(T):
                    while si < len(sched) and sched[si][0] <= t:
                        sched[si][1]()
                        si += 1
                    for c in chains:
                        c.step(t)
                while si < len(sched):
                    sched[si][1]()
                    si += 1

            # ---------- phase A: L0 fwd + bwd interleaved -----------------
            for fn in gemm_items_A("A", eT1, wih1_s, biasr1_s, 0):
                fn()
            for fn in gemm_items_A("B", eT2, wih2_s, biasr2_s, 0):
                fn()
            whh1_s = load(whh1, [128, 4, G])
            whh2_s = load(whh2, [128, 4, G])
            chA = Chain("A", whh1_s, nbc1_s, y=y1s, rev=False,
                        qp_split=True)
            chB = Chain("B", whh2_s, nbc2_s, y=y2s, rev=True,
                        qp_split=True)
            schedA = []
            for c in range(1, NBLK):
                schedA += paced(
                    gemm_items_A("A", eT1, wih1_s, biasr1_s, c)
                    + gemm_items_A("B", eT2, wih2_s, biasr2_s, c),
                    max(0, 16 * c - 44), 16 * c - 8,
                )
            run_phase([chA, chB], schedA)

            # ---------- phase B: L1 (merged xpL GEMM) ---------------------
            for fn in gemm_items_L(0):
                fn()
            chL = Chain("L", whhL_s, nbcL_s, use_pool=True,
                        split_sigma=True, qp_split=True)
            schedB = []
            for c in range(1, NBLK):
                schedB += paced(
                    gemm_items_L(c), max(0, 16 * c - 44), 16 * c - 8
                )
            run_phase([chL], schedB)

            # ---------- head ----------------------------------------------
            hd = psg.tile([128, 4, 16], f32, tag="pstn")
            for k in range(4):
                nc.tensor.matmul(
                    hd[:, 0, :], w1h_s[:, k, :], pooled[:, k, :],
                    start=(k == 0), stop=(k == 3),
                )
            ho = io.tile([128, 16], f32, tag="ho")
            nc.vector.tensor_copy(out=ho[:], in_=hd[:, 0, :])
            nc.sync.dma_start(out=headout[:], in_=ho[:])
            if DBG:
                nc.sync.dma_start(out=y1o[:], in_=y1s[:])
                nc.sync.dma_start(out=y2o[:], in_=y2s[:])
                nc.sync.dma_start(out=pooledo[:], in_=pooled[:])

    _split_multiwaits(nc, mybir)
    try:
        ents = getattr(tc, "_perfetto_entries", None)
        span = None
        if ents:
            starts = [e[1] for e in ents if e[1] is not None]
            ends = [e[2] if e[2] is not None else e[1] for e in ents]
            if starts and ends:
                span = int(max(ends) - min(starts))
        _CACHE["model_ns"] = span
    except Exception:
        _CACHE["model_ns"] = None
    return nc


def _prep_core_inputs(inputs, g, role):
    """Host-side sharding/layout prep for core (batch group g, role)."""
    f16 = np.float16
    x = np.asarray(inputs["x"]).astype(np.int64)
    emb = np.asarray(inputs["emb"], dtype=np.float32)
    embp = np.zeros((V, EP), dtype=np.float32)
    embp[:, :E] = emb

    xg = x[g * BL : (g + 1) * BL]
    e = embp[xg]
    eT_f = np.ascontiguousarray(e.transpose(2, 1, 0).reshape(EP, NTOK))
    er = e[:, ::-1, :]
    eT_r = np.ascontiguousarray(er.transpose(2, 1, 0).reshape(EP, NTOK))

    def ktile(wT, kt):
        Kd, Gd = wT.shape
        assert Kd == kt * 128
        return np.ascontiguousarray(
            wT.reshape(kt, 128, Gd).transpose(1, 0, 2)
        ).astype(f16)

    def e3(eT):
        return np.ascontiguousarray(
            eT.reshape(3, 128, NTOK).transpose(1, 0, 2)
        ).astype(f16)

    def biascols(b_ih, b_hh):
        bv = b_ih.copy()
        bv[: 2 * H] += b_hh[: 2 * H]
        rows = bv.reshape(1, 12, 128).astype(f16)
        nbc = np.repeat(
            np.ascontiguousarray(b_hh[2 * H :].reshape(4, 128).T)[:, :, None],
            16, axis=2,
        ).astype(f16)
        return rows, nbc

    w_ih0 = np.asarray(inputs["w_ih0"], dtype=np.float32)
    w_hh0 = np.asarray(inputs["w_hh0"], dtype=np.float32)
    b_ih0 = np.asarray(inputs["b_ih0"], dtype=np.float32)
    b_hh0 = np.asarray(inputs["b_hh0"], dtype=np.float32)
    w_ih1 = np.asarray(inputs["w_ih1"], dtype=np.float32)
    w_hh1 = np.asarray(inputs["w_hh1"], dtype=np.float32)
    b_ih1 = np.asarray(inputs["b_ih1"], dtype=np.float32)
    b_hh1 = np.asarray(inputs["b_hh1"], dtype=np.float32)
    w1 = np.asarray(inputs["w1"], dtype=np.float32)

    d1, d2 = (0, 1) if role == 0 else (1, 0)
    dL = role
    own_half = slice(0, H) if role == 0 else slice(H, 2 * H)
    oth_half = slice(H, 2 * H) if role == 0 else slice(0, H)

    def wihT(d):
        w = np.zeros((G, EP), dtype=np.float32)
        w[:, :E] = w_ih0[d]
        return ktile(w.T, 3)

    b1c, n1c = biascols(b_ih0[d1], b_hh0[d1])
    b2c, n2c = biascols(b_ih0[d2], b_hh0[d2])
    bLc, nLc = biascols(b_ih1[dL], b_hh1[dL])

    m = {
        "eT1": e3(eT_f if role == 0 else eT_r),
        "eT2": e3(eT_r if role == 0 else eT_f),
        "wih1": wihT(d1),
        "wih2": wihT(d2),
        "whh1": ktile(w_hh0[d1].T, 4),
        "whh2": ktile(w_hh0[d2].T, 4),
        "wa": ktile(w_ih1[dL][:, own_half].T, 4),
        "wb": ktile(w_ih1[dL][:, oth_half].T, 4),
        "whhL": ktile(w_hh1[dL].T, 4),
        "biasr1": b1c, "biasr2": b2c, "biasrL": bLc,
        "ones": np.ones((1, 256), dtype=f16),
        "nbc1": n1c, "nbc2": n2c, "nbcL": nLc,
        "ident": np.eye(128, dtype=f16),
        "w1h": ktile(w1[:, own_half].T, 4),
    }
    return m


def kernel(**inputs) -> np.ndarray:
    from concourse.bass_utils import run_bass_kernel_spmd

    if "nc" not in _CACHE:
        _CACHE["nc"] = _build_nc()
    nc = _CACHE["nc"]

    core_ids = list(range(8))
    in_maps = []
    for c in core_ids:
        g, role = c % 4, c // 4
        in_maps.append(_prep_core_inputs(inputs, g, role))

    res = run_bass_kernel_spmd(nc, in_maps, core_ids)
    _CACHE["last_res"] = res

    b1 = np.asarray(inputs["b1"], dtype=np.float32)
    w2 = np.asarray(inputs["w2"], dtype=np.float32)
    b2 = np.asarray(inputs["b2"], dtype=np.float32)
    out = np.zeros((B, 2), dtype=np.float32)
    for g in range(4):
        p = (
            res.results[g]["headout"].astype(np.float32)
            + res.results[g + 4]["headout"].astype(np.float32)
        )
        hid = np.maximum(p + b1[:, None], 0.0)
        logits = w2 @ hid + b2[:, None]
        out[g * BL : (g + 1) * BL] = logits.T
    return out


# revision 41
# speedup vs baseline: 2.4956x; 1.0402x over previous
"""Bass/Trainium2 kernel for nn_GRUClassifier: 2-layer BiGRU + max-pool + MLP head.

Sharding: 8 cores = 4 batch groups x 2 L1-direction roles (as baseline).
Each core computes BOTH L0 directions for its 16 sequences, then one L1
direction, max-pool, and the W1 partial; host sums role-pair partials and
applies relu + W2.

Perf design (cost-model driven; the recurrence is dependency-chain-bound):
- The two L0 recurrences run INTERLEAVED step-by-step (independent chains)
  so engine latency of one chain hides under the other.
- Per-step op count: 5 DVE + 2 Act:
  * xp and the n-gate b_hh bias are injected into PSUM with identity
    matmuls (PE is nearly free), removing the gate-sum DVE ops.
    Accumulation groups are kept CONTIGUOUS per PSUM region (interleaved
    start-groups within a bank corrupt accumulation).
  * r,z sigmoids fused into one activation over [128,8,16]; separate PSUM
    tiles for rz vs n gates so the sigmoid does not wait on n matmuls
    (Tile tracks deps at whole-tile granularity).
  * h update: h = z*h + (1-z)*n with q=z*h computed off-chain and
    p=(z-1)*(-n) from the negated tanh output (z kept f32 to avoid
    cancellation in 1-z).
- xp blocks live entirely in SBUF ring tiles: the input-projection GEMMs
  write them directly (no DRAM roundtrip, no DMA races), sliced into small
  items paced between recurrence steps.
- y (L0 outputs) stay SBUF-resident; the backward chain writes y in
  position-reversed slots so BOTH halves of the L1 input GEMM accumulate
  in one PSUM pass (single xpL stream; no per-step peer adds).
"""
import os
import sys
import numpy as np

sys.path.insert(0, "/opt/trn_rl_repo")

B, T, E, H, V = 64, 256, 300, 512, 50000
EP = 384            # E padded to 3*128
G = 3 * H           # 1536 gate rows = 12 chunks of 128
BL = 16             # batch per core
NTOK = T * BL       # 4096
NBLK = 16           # 256-col xp blocks (16 steps each)

_CACHE = {}


def _patch_drain():
    """walrus CoreV3 rejects CTRL (Drain) instructions with too many sem
    waits; split the tail-drain's waits across preceding sync nops."""
    from concourse import mybir
    from concourse.tile import TileContext
    from concourse.vector_clock import ScopedClock

    if getattr(TileContext, "_drain_patched", False):
        return
    MAXW = 1

    def _drain_and_barrier(self, tick_clock, wait_clock):
        drain_inst = self.nc.sync.drain()
        wait_clock.add_sem_waits(
            drain_inst.ins, ScopedClock({None: tick_clock.global_clock})
        )
        si = drain_inst.ins.sync_info
        if si is not None and si.on_wait and len(si.on_wait) > MAXW:
            waits = list(si.on_wait)
            si.on_wait = waits[:MAXW]
            for i in range(MAXW, len(waits), MAXW):
                nop = self.nc.sync.nop(nofuse=True, hint="drain_wait_split")
                nsi = nop.ins.sync_info
                if nsi is None:
                    nop.ins.sync_info = mybir.SyncInfo(
                        on_wait=waits[i : i + MAXW], on_update=[]
                    )
                else:
                    nsi.on_wait = waits[i : i + MAXW]
        self.nc.all_engine_barrier()
        assert self.sems is not None
        popped = self.nc._tile_sem_poison_stack.pop()
        assert popped is self._sem_poison
        self.nc.clear_and_free_semaphores(list(self.sems.allocated().values()))
        self.nc.all_engine_barrier()

    TileContext._drain_and_barrier = _drain_and_barrier
    TileContext._drain_patched = True


def _split_multiwaits(nc, mybir, maxw=1):
    """walrus CoreV2/V3 setupSyncWait rejects instructions with more than one
    sem wait; split extras onto preceding same-engine nops."""
    cnt = 0
    for fn in nc.m.functions:
        for bb in fn.blocks:
            insts = bb.instructions
            out = []
            changed = False
            for inst in insts:
                si = getattr(inst, "sync_info", None)
                eng = getattr(inst, "engine", None)
                if (
                    si is not None
                    and si.on_wait
                    and len(si.on_wait) > maxw
                    and eng is not None
                    and eng != mybir.EngineType.Unassigned
                ):
                    waits = list(si.on_wait)
                    for w in waits[:-maxw]:
                        nop = mybir.InstNoOp(
                            name=f"ws_nop_{cnt}", ins=[], outs=[]
                        )
                        cnt += 1
                        nop.engine = eng
                        nop.sync_info = mybir.SyncInfo(
                            on_wait=[w], on_update=[]
                        )
                        out.append(nop)
                    si.on_wait = waits[-maxw:]
                    changed = True
                out.append(inst)
            if changed:
                bb.instructions = out


def _build_nc():
    from concourse import bass, mybir
    from concourse.tile import TileContext

    _patch_drain()
    f16 = mybir.dt.float16
    f32 = mybir.dt.float32
    AF = mybir.ActivationFunctionType
    OP = mybir.AluOpType

    nc = bass.Bass(target_bir_lowering=False)

    def par(name, shape, dt=f16, out=False):
        return nc.declare_dram_parameter(name, list(shape), dt, isOutput=out)

    eT1 = par("eT1", [128, 3, NTOK])
    eT2 = par("eT2", [128, 3, NTOK])
    wih1 = par("wih1", [128, 3, G])
    wih2 = par("wih2", [128, 3, G])
    whh1 = par("whh1", [128, 4, G])
    whh2 = par("whh2", [128, 4, G])
    wa = par("wa", [128, 4, G])
    wb = par("wb", [128, 4, G])
    whhL = par("whhL", [128, 4, G])
    biasrL = par("biasrL", [1, 12, 128])
    ones = par("ones", [1, 256])
    nbc1 = par("nbc1", [128, 4, 16])
    nbc2 = par("nbc2", [128, 4, 16])
    nbcL = par("nbcL", [128, 4, 16])
    ident = par("ident", [128, 128])
    w1h = par("w1h", [128, 4, 128])
    headout = par("headout", [128, 16], f32, out=True)
    DBG = os.environ.get("GRU_DEBUG_DUMP") == "1"
    if DBG:
        y1o = par("y1o", [128, 4, NTOK], out=True)
        y2o = par("y2o", [128, 4, NTOK], out=True)
        pooledo = par("pooledo", [128, 4, 16], out=True)

    with TileContext(nc) as tc:
        with (
            tc.tile_pool(name="wpool", bufs=1) as wp,
            tc.tile_pool(name="io", bufs=3) as io,
            tc.tile_pool(name="xpb", bufs=6) as xpb,
            tc.tile_pool(name="ew", bufs=2) as ew,
            tc.tile_pool(name="hp", bufs=2) as hp,
            tc.tile_pool(name="ps", bufs=2, space="PSUM") as ps,
            tc.tile_pool(name="psg", bufs=3, space="PSUM") as psg,
        ):
            def load(p, shape, dt=f16):
                t = wp.tile(list(shape), dt, tag=p.name + "_sb")
                nc.sync.dma_start(out=t[:], in_=p[:])
                return t

            # phase-A-critical weights first (SP DMA queue is in-order)
            wih1_s = load(wih1, [128, 3, G])
            wih2_s = load(wih2, [128, 3, G])
            ones_s = load(ones, [1, 256])
            ident_s = load(ident, [128, 128])
            nbc1_s = load(nbc1, [128, 4, 16])
            nbc2_s = load(nbc2, [128, 4, 16])

            def load_late(p, shape, dt=f16):
                # phase-B weights ride the idle GPSIMD DGE queue
                t = wp.tile(list(shape), dt, tag=p.name + "_sb")
                nc.gpsimd.dma_start(out=t[:], in_=p[:])
                return t

            wa_s = load_late(wa, [128, 4, G])
            wb_s = load_late(wb, [128, 4, G])
            whhL_s = load_late(whhL, [128, 4, G])
            biasrL_s = load_late(biasrL, [1, 12, 128])
            nbcL_s = load_late(nbcL, [128, 4, 16])
            w1h_s = load_late(w1h, [128, 4, 128])

            y1s = wp.tile([128, 4, NTOK], f16, tag="y1s")
            y2s = wp.tile([128, 4, NTOK], f16, tag="y2s")
            h0 = wp.tile([128, 4, 16], f16, tag="h0")
            nc.vector.memset(h0[:], 0.0)
            pooled = wp.tile([128, 4, 16], f16, tag="pooled")
            nc.vector.memset(pooled[:], -60000.0)

            ring = {}

            # ---------- GEMM item generators (finely sliced side work) ----
            # Each 256-col xp block is computed straight into an SBUF ring
            # tile; items are individually small so they pace between steps.
            # Bias is folded into the GEMM via a K=1 bias-row matmul, so the
            # PSUM->SBUF downcast is a bias-free Copy fused over 2 m-chunks
            # (fewer, Act-only fin ops keep the DVE queue clear for chains).
            def gemm_items_A(cid, eT_dram, w_sb, blk):
                sl = slice(blk * 256, (blk + 1) * 256)
                st = {}
                items = []

                def open_():
                    ring[(cid, blk)] = xpb.tile(
                        [128, 12, 256], f16, tag="xpb", name="xpblk")
                    et = io.tile([128, 3, 256], f16, tag="et_in")
                    nc.sync.dma_start(out=et[:], in_=eT_dram[:, :, sl])
                    st["et"] = et

                items.append(open_)
                for m2 in range(6):
                    def mk(m2):
                        def mmsa():
                            p = ps.tile([128, 2, 256], f32, tag="gps")
                            st[m2] = p
                            m = 2 * m2
                            for k in range(3):
                                nc.tensor.matmul(
                                    p[:, 0, :],
                                    w_sb[:, k, m * 128 : (m + 1) * 128],
                                    st["et"][:, k, :],
                                    start=(k == 0), stop=(k == 2),
                                )

                        def mmsb():
                            p = st[m2]
                            m = 2 * m2 + 1
                            for k in range(3):
                                nc.tensor.matmul(
                                    p[:, 1, :],
                                    w_sb[:, k, m * 128 : (m + 1) * 128],
                                    st["et"][:, k, :],
                                    start=(k == 0), stop=(k == 2),
                                )

                        def fin():
                            p = st.pop(m2)
                            xb = ring[(cid, blk)]
                            nc.scalar.activation(
                                xb[:, 2 * m2 : 2 * m2 + 2, :], p[:], AF.Copy,
                            )

                        return mmsa, mmsb, fin

                    items += list(mk(m2))
                return items

            def gemm_items_L(blk):
                sl = slice(blk * 256, (blk + 1) * 256)
                st = {}
                items = []

                def open_():
                    ring[("L", blk)] = xpb.tile(
                        [128, 12, 256], f16, tag="xpb", name="xpblk")

                items.append(open_)
                for m2 in range(6):
                    def mk(m2):
                        def half(m, pi, p):
                            for k in range(4):
                                nc.tensor.matmul(
                                    p[:, pi, :],
                                    wa_s[:, k, m * 128 : (m + 1) * 128],
                                    y1s[:, k, sl],
                                    start=(k == 0), stop=False,
                                )
                            for k in range(4):
                                nc.tensor.matmul(
                                    p[:, pi, :],
                                    wb_s[:, k, m * 128 : (m + 1) * 128],
                                    y2s[:, k, sl],
                                    start=False, stop=False,
                                )
                            nc.tensor.matmul(
                                p[:, pi, :], biasrL_s[0:1, m, :],
                                ones_s[0:1, :], start=False, stop=True,
                            )

                        def mmsa():
                            p = ps.tile([128, 2, 256], f32, tag="gps")
                            st[m2] = p
                            half(2 * m2, 0, p)

                        def mmsb():
                            half(2 * m2 + 1, 1, st[m2])

                        def fin():
                            p = st.pop(m2)
                            xb = ring[("L", blk)]
                            nc.scalar.activation(
                                xb[:, 2 * m2 : 2 * m2 + 2, :], p[:], AF.Copy,
                            )

                        return mmsa, mmsb, fin

                    items += list(mk(m2))
                return items

            # ---------- recurrence chain -----------------------------------
            class Chain:
                def __init__(self, cid, whh_sb, nbc_sb,
                             y=None, rev=False, use_pool=False,
                             split_sigma=False, qp_split=False):
                    self.cid = cid
                    self.whh = whh_sb
                    self.nbc = nbc_sb
                    self.y = y
                    self.rev = rev
                    self.use_pool = use_pool
                    self.split_sigma = split_sigma
                    self.qp_split = qp_split
                    self.cur = None
                    self.hprev = h0
                    self.qprev = None
                    self.pprev = None

                def _col(self, t):
                    c = (T - 1 - t) if self.rev else t
                    return slice(c * 16, (c + 1) * 16)

                def step(self, t):
                    v = t % 16
                    if v == 0:
                        self.cur = ring.pop((self.cid, t // 16))
                    cur = self.cur
                    vs = slice(v * 16, (v + 1) * 16)
                    pn = psg.tile([128, 4, 16], f32, tag="pstn")
                    hp_ = self.hprev

                    def hrhs(k):
                        return (
                            hp_[:, k, :] if not isinstance(hp_, tuple)
                            else hp_[0][:, k, hp_[1]]
                        )

                    # One ungated injection matmul per PSUM tile opens that
                    # bank's single accumulation group (it runs ahead of the
                    # h-gated whh matmuls on the in-order PE); all whh
                    # matmuls then accumulate region-wise inside the open
                    # group, closing it on the last one.
                    def inj(dst, src):
                        nc.tensor.matmul(dst, ident_s[:], src,
                                         start=True, stop=False)

                    def whh_mms(dst, m0, m1, rhs=None, stop=True):
                        for m in range(m0, m1):
                            for k in range(4):
                                nc.tensor.matmul(
                                    dst[:, m - m0, :],
                                    self.whh[:, k, m * 128 : (m + 1) * 128],
                                    hrhs(k) if rhs is None else rhs[:, k, :],
                                    start=False,
                                    stop=(stop and m == m1 - 1 and k == 3),
                                )

                    if self.qp_split and self.split_sigma:
                        # gh = Whh*q + Whh*p (exact); the q half fires as
                        # soon as q(t-1) exists (mid-chain), only the p half
                        # gates the sigmoid -> h drops off the critical path.
                        pr = psg.tile([128, 4, 16], f32, tag="pstrz",
                                      name="pr")
                        pz = psg.tile([128, 4, 16], f32, tag="pstrz",
                                      name="pz")
                        if t == 0:
                            nc.tensor.matmul(pn[:], ident_s[:], self.nbc[:],
                                             start=True, stop=True)
                            nc.tensor.matmul(pr[:], ident_s[:],
                                             cur[:, 0:4, vs],
                                             start=True, stop=True)
                            nc.tensor.matmul(pz[:], ident_s[:],
                                             cur[:, 4:8, vs],
                                             start=True, stop=True)
                        else:
                            inj(pn[:], self.nbc[:])
                            inj(pr[:], cur[:, 0:4, vs])
                            inj(pz[:], cur[:, 4:8, vs])
                            qq, pp = self.qprev, self.pprev
                            whh_mms(pr, 0, 4, rhs=qq, stop=False)
                            whh_mms(pz, 4, 8, rhs=qq, stop=False)
                            whh_mms(pr, 0, 4, rhs=pp)
                            whh_mms(pz, 4, 8, rhs=pp)
                            whh_mms(pn, 8, 12)
                        rt = ew.tile([128, 4, 16], f32, tag=f"rt{self.cid}")
                        zt = ew.tile([128, 4, 16], f32, tag=f"zt{self.cid}")
                        nc.scalar.activation(rt[:], pr[:], AF.Sigmoid)
                        nc.scalar.activation(zt[:], pz[:], AF.Sigmoid)
                        rr, zz = rt[:], zt[:]
                    elif self.qp_split:
                        prz = psg.tile([128, 8, 16], f32, tag="pstrz",
                                       name="prz")
                        if t == 0:
                            nc.tensor.matmul(pn[:], ident_s[:], self.nbc[:],
                                             start=True, stop=True)
                            nc.tensor.matmul(prz[:], ident_s[:],
                                             cur[:, 0:8, vs],
                                             start=True, stop=True)
                        else:
                            inj(pn[:], self.nbc[:])
                            inj(prz[:], cur[:, 0:8, vs])
                            qq, pp = self.qprev, self.pprev
                            whh_mms(prz, 0, 8, rhs=qq, stop=False)
                            whh_mms(prz, 0, 8, rhs=pp)
                            whh_mms(pn, 8, 12)
                        rz = ew.tile([128, 8, 16], f32, tag=f"rz{self.cid}")
                        nc.scalar.activation(rz[:], prz[:], AF.Sigmoid)
                        rr, zz = rz[:, 0:4, :], rz[:, 4:8, :]
                    elif self.split_sigma:
                        inj(pn[:], self.nbc[:])
                        pr = psg.tile([128, 4, 16], f32, tag="pstrz",
                                      name="pr")
                        pz = psg.tile([128, 4, 16], f32, tag="pstrz",
                                      name="pz")
                        inj(pr[:], cur[:, 0:4, vs])
                        inj(pz[:], cur[:, 4:8, vs])
                        whh_mms(pr, 0, 4)
                        whh_mms(pz, 4, 8)
                        whh_mms(pn, 8, 12)
                        rt = ew.tile([128, 4, 16], f32, tag=f"rt{self.cid}")
                        zt = ew.tile([128, 4, 16], f32, tag=f"zt{self.cid}")
                        nc.scalar.activation(rt[:], pr[:], AF.Sigmoid)
                        nc.scalar.activation(zt[:], pz[:], AF.Sigmoid)
                        rr, zz = rt[:], zt[:]
                    else:
                        inj(pn[:], self.nbc[:])
                        prz = psg.tile([128, 8, 16], f32, tag="pstrz",
                                       name="prz")
                        inj(prz[:], cur[:, 0:8, vs])
                        whh_mms(prz, 0, 8)
                        whh_mms(pn, 8, 12)
                        rz = ew.tile([128, 8, 16], f32, tag=f"rz{self.cid}")
                        nc.scalar.activation(rz[:], prz[:], AF.Sigmoid)
                        rr, zz = rz[:, 0:4, :], rz[:, 4:8, :]
                    u = ew.tile([128, 4, 16], f32, tag=f"u{self.cid}")
                    nc.vector.scalar_tensor_tensor(
                        out=u[:], in0=pn[:], scalar=1.0,
                        in1=rr, op0=OP.mult, op1=OP.mult,
                    )
                    tn = ew.tile([128, 4, 16], f32, tag=f"tn{self.cid}")
                    nc.vector.scalar_tensor_tensor(
                        out=tn[:], in0=u[:], scalar=1.0,
                        in1=cur[:, 8:12, vs], op0=OP.mult, op1=OP.add,
                    )
                    nn = ew.tile([128, 4, 16], f16, tag=f"nn{self.cid}")
                    nc.scalar.activation(nn[:], tn[:], AF.Tanh, scale=-1.0)
                    q = ew.tile([128, 4, 16], f16, tag=f"q{self.cid}")
                    nc.vector.scalar_tensor_tensor(
                        out=q[:], in0=zz, scalar=1.0,
                        in1=hp_[:] if not isinstance(hp_, tuple)
                        else hp_[0][:, :, hp_[1]],
                        op0=OP.mult, op1=OP.mult,
                    )
                    p = ew.tile([128, 4, 16], f16, tag=f"p{self.cid}")
                    nc.vector.scalar_tensor_tensor(
                        out=p[:], in0=zz, scalar=1.0,
                        in1=nn[:], op0=OP.subtract, op1=OP.mult,
                    )
                    if self.y is not None:
                        cs = self._col(t)
                        nc.vector.scalar_tensor_tensor(
                            out=self.y[:, :, cs], in0=q[:], scalar=1.0,
                            in1=p[:], op0=OP.mult, op1=OP.add,
                        )
                        self.hprev = (self.y, cs)
                    else:
                        hn = hp.tile([128, 4, 16], f16, tag="hL")
                        nc.vector.scalar_tensor_tensor(
                            out=hn[:], in0=q[:], scalar=1.0,
                            in1=p[:], op0=OP.mult, op1=OP.add,
                        )
                        if self.use_pool:
                            nc.vector.scalar_tensor_tensor(
                                out=pooled[:], in0=hn[:], scalar=1.0,
                                in1=pooled[:], op0=OP.mult, op1=OP.max,
                            )
                        self.hprev = hn
                    self.qprev = q
                    self.pprev = p

            # ---------- item pacing ---------------------------------------
            def paced(items, t0, t1):
                n = len(items)
                w = max(1, t1 - t0)
                return [(t0 + (i * w) // n, fn) for i, fn in enumerate(items)]

            def run_phase(chains, sched):
                sched = sorted(sched, key=lambda x: x[0])
                si = 0
                for t in range(T):
                    while si < len(sched) and sched[si][0] <= t:
                        sched[si][1]()
                        si += 1
                    for c in chains:
                        c.step(t)
                while si < len(sched):
                    sched[si][1]()
                    si += 1

            # ---------- phase A: L0 fwd + bwd interleaved -----------------
            for fn in gemm_items_A("A", eT1, wih1_s, 0):
                fn()
            for fn in gemm_items_A("B", eT2, wih2_s, 0):
                fn()
            whh1_s = load(whh1, [128, 4, G])
            whh2_s = load(whh2, [128, 4, G])
            chA = Chain("A", whh1_s, nbc1_s, y=y1s, rev=False,
                        qp_split=True)
            chB = Chain("B", whh2_s, nbc2_s, y=y2s, rev=True,
                        qp_split=True)
            schedA = []
            for c in range(1, NBLK):
                schedA += paced(
                    gemm_items_A("A", eT1, wih1_s, c)
                    + gemm_items_A("B", eT2, wih2_s, c),
                    max(0, 16 * c - 44), 16 * c - 8,
                )
            run_phase([chA, chB], schedA)

            # ---------- phase B: L1 (merged xpL GEMM) ---------------------
            for fn in gemm_items_L(0):
                fn()
            chL = Chain("L", whhL_s, nbcL_s, use_pool=True,
                        split_sigma=True, qp_split=True)
            schedB = []
            for c in range(1, NBLK):
                schedB += paced(
                    gemm_items_L(c), max(0, 16 * c - 44), 16 * c - 8
                )
            run_phase([chL], schedB)

            # ---------- head ----------------------------------------------
            hd = psg.tile([128, 4, 16], f32, tag="pstn")
            for k in range(4):
                nc.tensor.matmul(
                    hd[:, 0, :], w1h_s[:, k, :], pooled[:, k, :],
                    start=(k == 0), stop=(k == 3),
                )
            ho = io.tile([128, 16], f32, tag="ho")
            nc.vector.tensor_copy(out=ho[:], in_=hd[:, 0, :])
            nc.sync.dma_start(out=headout[:], in_=ho[:])
            if DBG:
                nc.sync.dma_start(out=y1o[:], in_=y1s[:])
                nc.sync.dma_start(out=y2o[:], in_=y2s[:])
                nc.sync.dma_start(out=pooledo[:], in_=pooled[:])

    _split_multiwaits(nc, mybir)
    try:
        ents = getattr(tc, "_perfetto_entries", None)
        span = None
        if ents:
            starts = [e[1] for e in ents if e[1] is not None]
            ends = [e[2] if e[2] is not None else e[1] for e in ents]
            if starts and ends:
                span = int(max(ends) - min(starts))
        _CACHE["model_ns"] = span
    except Exception:
        _CACHE["model_ns"] = None
    return nc


def _prep_core_inputs(inputs, g, role):
    """Host-side sharding/layout prep for core (batch group g, role)."""
    f16 = np.float16
    x = np.asarray(inputs["x"]).astype(np.int64)
    emb = np.asarray(inputs["emb"], dtype=np.float32)
    embp = np.zeros((V, EP), dtype=np.float32)
    embp[:, :E] = emb
    embp[:, E] = 1.0

    xg = x[g * BL : (g + 1) * BL]
    e = embp[xg]
    eT_f = np.ascontiguousarray(e.transpose(2, 1, 0).reshape(EP, NTOK))
    er = e[:, ::-1, :]
    eT_r = np.ascontiguousarray(er.transpose(2, 1, 0).reshape(EP, NTOK))

    def ktile(wT, kt):
        Kd, Gd = wT.shape
        assert Kd == kt * 128
        return np.ascontiguousarray(
            wT.reshape(kt, 128, Gd).transpose(1, 0, 2)
        ).astype(f16)

    def e3(eT):
        return np.ascontiguousarray(
            eT.reshape(3, 128, NTOK).transpose(1, 0, 2)
        ).astype(f16)

    def biascols(b_ih, b_hh):
        bv = b_ih.copy()
        bv[: 2 * H] += b_hh[: 2 * H]
        rows = bv.reshape(1, 12, 128).astype(f16)
        nbc = np.repeat(
            np.ascontiguousarray(b_hh[2 * H :].reshape(4, 128).T)[:, :, None],
            16, axis=2,
        ).astype(f16)
        return rows, nbc

    w_ih0 = np.asarray(inputs["w_ih0"], dtype=np.float32)
    w_hh0 = np.asarray(inputs["w_hh0"], dtype=np.float32)
    b_ih0 = np.asarray(inputs["b_ih0"], dtype=np.float32)
    b_hh0 = np.asarray(inputs["b_hh0"], dtype=np.float32)
    w_ih1 = np.asarray(inputs["w_ih1"], dtype=np.float32)
    w_hh1 = np.asarray(inputs["w_hh1"], dtype=np.float32)
    b_ih1 = np.asarray(inputs["b_ih1"], dtype=np.float32)
    b_hh1 = np.asarray(inputs["b_hh1"], dtype=np.float32)
    w1 = np.asarray(inputs["w1"], dtype=np.float32)

    d1, d2 = (0, 1) if role == 0 else (1, 0)
    dL = role
    own_half = slice(0, H) if role == 0 else slice(H, 2 * H)
    oth_half = slice(H, 2 * H) if role == 0 else slice(0, H)

    def wihT(d, bv):
        w = np.zeros((G, EP), dtype=np.float32)
        w[:, :E] = w_ih0[d]
        w[:, E] = bv
        return ktile(w.T, 3)

    def biasvec(b_ih, b_hh):
        bv = b_ih.copy()
        bv[: 2 * H] += b_hh[: 2 * H]
        return bv

    bv1 = biasvec(b_ih0[d1], b_hh0[d1])
    bv2 = biasvec(b_ih0[d2], b_hh0[d2])
    _, n1c = biascols(b_ih0[d1], b_hh0[d1])
    _, n2c = biascols(b_ih0[d2], b_hh0[d2])
    bLc, nLc = biascols(b_ih1[dL], b_hh1[dL])

    m = {
        "eT1": e3(eT_f if role == 0 else eT_r),
        "eT2": e3(eT_r if role == 0 else eT_f),
        "wih1": wihT(d1, bv1),
        "wih2": wihT(d2, bv2),
        "whh1": ktile(w_hh0[d1].T, 4),
        "whh2": ktile(w_hh0[d2].T, 4),
        "wa": ktile(w_ih1[dL][:, own_half].T, 4),
        "wb": ktile(w_ih1[dL][:, oth_half].T, 4),
        "whhL": ktile(w_hh1[dL].T, 4),
        "biasrL": bLc,
        "ones": np.ones((1, 256), dtype=f16),
        "nbc1": n1c, "nbc2": n2c, "nbcL": nLc,
        "ident": np.eye(128, dtype=f16),
        "w1h": ktile(w1[:, own_half].T, 4),
    }
    return m


def kernel(**inputs) -> np.ndarray:
    from concourse.bass_utils import run_bass_kernel_spmd

    if "nc" not in _CACHE:
        _CACHE["nc"] = _build_nc()
    nc = _CACHE["nc"]

    core_ids = list(range(8))
    in_maps = []
    for c in core_ids:
        g, role = c % 4, c // 4
        in_maps.append(_prep_core_inputs(inputs, g, role))

    res = run_bass_kernel_spmd(nc, in_maps, core_ids)
    _CACHE["last_res"] = res

    b1 = np.asarray(inputs["b1"], dtype=np.float32)
    w2 = np.asarray(inputs["w2"], dtype=np.float32)
    b2 = np.asarray(inputs["b2"], dtype=np.float32)
    out = np.zeros((B, 2), dtype=np.float32)
    for g in range(4):
        p = (
            res.results[g]["headout"].astype(np.float32)
            + res.results[g + 4]["headout"].astype(np.float32)
        )
        hid = np.maximum(p + b1[:, None], 0.0)
        logits = w2 @ hid + b2[:, None]
        out[g * BL : (g + 1) * BL] = logits.T
    return out


# revision 48
# speedup vs baseline: 2.5011x; 1.0022x over previous
"""Bass/Trainium2 kernel for nn_GRUClassifier: 2-layer BiGRU + max-pool + MLP head.

Sharding: 8 cores = 4 batch groups x 2 L1-direction roles (as baseline).
Each core computes BOTH L0 directions for its 16 sequences, then one L1
direction, max-pool, and the W1 partial; host sums role-pair partials and
applies relu + W2.

Perf design (cost-model driven; the recurrence is dependency-chain-bound):
- The two L0 recurrences run INTERLEAVED step-by-step (independent chains)
  so engine latency of one chain hides under the other.
- Per-step op count: 5 DVE + 2 Act:
  * xp and the n-gate b_hh bias are injected into PSUM with identity
    matmuls (PE is nearly free), removing the gate-sum DVE ops.
    Accumulation groups are kept CONTIGUOUS per PSUM region (interleaved
    start-groups within a bank corrupt accumulation).
  * r,z sigmoids fused into one activation over [128,8,16]; separate PSUM
    tiles for rz vs n gates so the sigmoid does not wait on n matmuls
    (Tile tracks deps at whole-tile granularity).
  * h update: h = z*h + (1-z)*n with q=z*h computed off-chain and
    p=(z-1)*(-n) from the negated tanh output (z kept f32 to avoid
    cancellation in 1-z).
- xp blocks live entirely in SBUF ring tiles: the input-projection GEMMs
  write them directly (no DRAM roundtrip, no DMA races), sliced into small
  items paced between recurrence steps.
- y (L0 outputs) stay SBUF-resident; the backward chain writes y in
  position-reversed slots so BOTH halves of the L1 input GEMM accumulate
  in one PSUM pass (single xpL stream; no per-step peer adds).
"""
import os
import sys
import numpy as np

sys.path.insert(0, "/opt/trn_rl_repo")

B, T, E, H, V = 64, 256, 300, 512, 50000
EP = 384            # E padded to 3*128
G = 3 * H           # 1536 gate rows = 12 chunks of 128
BL = 16             # batch per core
NTOK = T * BL       # 4096
NBLK = 16           # 256-col xp blocks (16 steps each)

_CACHE = {}


def _patch_drain():
    """walrus CoreV3 rejects CTRL (Drain) instructions with too many sem
    waits; split the tail-drain's waits across preceding sync nops."""
    from concourse import mybir
    from concourse.tile import TileContext
    from concourse.vector_clock import ScopedClock

    if getattr(TileContext, "_drain_patched", False):
        return
    MAXW = 1

    def _drain_and_barrier(self, tick_clock, wait_clock):
        drain_inst = self.nc.sync.drain()
        wait_clock.add_sem_waits(
            drain_inst.ins, ScopedClock({None: tick_clock.global_clock})
        )
        si = drain_inst.ins.sync_info
        if si is not None and si.on_wait and len(si.on_wait) > MAXW:
            waits = list(si.on_wait)
            si.on_wait = waits[:MAXW]
            for i in range(MAXW, len(waits), MAXW):
                nop = self.nc.sync.nop(nofuse=True, hint="drain_wait_split")
                nsi = nop.ins.sync_info
                if nsi is None:
                    nop.ins.sync_info = mybir.SyncInfo(
                        on_wait=waits[i : i + MAXW], on_update=[]
                    )
                else:
                    nsi.on_wait = waits[i : i + MAXW]
        self.nc.all_engine_barrier()
        assert self.sems is not None
        popped = self.nc._tile_sem_poison_stack.pop()
        assert popped is self._sem_poison
        self.nc.clear_and_free_semaphores(list(self.sems.allocated().values()))
        self.nc.all_engine_barrier()

    TileContext._drain_and_barrier = _drain_and_barrier
    TileContext._drain_patched = True


def _split_multiwaits(nc, mybir, maxw=1):
    """walrus CoreV2/V3 setupSyncWait rejects instructions with more than one
    sem wait; split extras onto preceding same-engine nops."""
    cnt = 0
    for fn in nc.m.functions:
        for bb in fn.blocks:
            insts = bb.instructions
            out = []
            changed = False
            for inst in insts:
                si = getattr(inst, "sync_info", None)
                eng = getattr(inst, "engine", None)
                if (
                    si is not None
                    and si.on_wait
                    and len(si.on_wait) > maxw
                    and eng is not None
                    and eng != mybir.EngineType.Unassigned
                ):
                    waits = list(si.on_wait)
                    for w in waits[:-maxw]:
                        nop = mybir.InstNoOp(
                            name=f"ws_nop_{cnt}", ins=[], outs=[]
                        )
                        cnt += 1
                        nop.engine = eng
                        nop.sync_info = mybir.SyncInfo(
                            on_wait=[w], on_update=[]
                        )
                        out.append(nop)
                    si.on_wait = waits[-maxw:]
                    changed = True
                out.append(inst)
            if changed:
                bb.instructions = out


def _build_nc():
    from concourse import bass, mybir
    from concourse.tile import TileContext

    _patch_drain()
    f16 = mybir.dt.float16
    f32 = mybir.dt.float32
    AF = mybir.ActivationFunctionType
    OP = mybir.AluOpType

    nc = bass.Bass(target_bir_lowering=False)

    def par(name, shape, dt=f16, out=False):
        return nc.declare_dram_parameter(name, list(shape), dt, isOutput=out)

    eT1 = par("eT1", [128, 3, NTOK])
    eT2 = par("eT2", [128, 3, NTOK])
    wih1 = par("wih1", [128, 3, G])
    wih2 = par("wih2", [128, 3, G])
    whh1 = par("whh1", [128, 4, G])
    whh2 = par("whh2", [128, 4, G])
    wa = par("wa", [128, 4, G])
    wb = par("wb", [128, 4, G])
    whhL = par("whhL", [128, 4, G])
    biasrL = par("biasrL", [1, 12, 128])
    ones = par("ones", [1, 256])
    nbc1 = par("nbc1", [128, 4, 16])
    nbc2 = par("nbc2", [128, 4, 16])
    nbcL = par("nbcL", [128, 4, 16])
    ident = par("ident", [128, 128])
    w1h = par("w1h", [128, 4, 128])
    headout = par("headout", [128, 16], f32, out=True)
    DBG = os.environ.get("GRU_DEBUG_DUMP") == "1"
    if DBG:
        y1o = par("y1o", [128, 4, NTOK], out=True)
        y2o = par("y2o", [128, 4, NTOK], out=True)
        pooledo = par("pooledo", [128, 4, 16], out=True)

    with TileContext(nc) as tc:
        with (
            tc.tile_pool(name="wpool", bufs=1) as wp,
            tc.tile_pool(name="io", bufs=3) as io,
            tc.tile_pool(name="xpb", bufs=6) as xpb,
            tc.tile_pool(name="ew", bufs=2) as ew,
            tc.tile_pool(name="hp", bufs=2) as hp,
            tc.tile_pool(name="ps", bufs=2, space="PSUM") as ps,
            tc.tile_pool(name="psg", bufs=3, space="PSUM") as psg,
        ):
            def load(p, shape, dt=f16):
                t = wp.tile(list(shape), dt, tag=p.name + "_sb")
                nc.sync.dma_start(out=t[:], in_=p[:])
                return t

            # phase-A-critical weights first (SP DMA queue is in-order)
            wih1_s = load(wih1, [128, 3, G])

            def load_late(p, shape, dt=f16):
                # phase-B weights ride the idle GPSIMD DGE queue
                t = wp.tile(list(shape), dt, tag=p.name + "_sb")
                nc.gpsimd.dma_start(out=t[:], in_=p[:])
                return t

            wa_s = load_late(wa, [128, 4, G])
            wb_s = load_late(wb, [128, 4, G])
            whhL_s = load_late(whhL, [128, 4, G])
            biasrL_s = load_late(biasrL, [1, 12, 128])
            nbcL_s = load_late(nbcL, [128, 4, 16])
            w1h_s = load_late(w1h, [128, 4, 128])

            y1s = wp.tile([128, 4, NTOK], f16, tag="y1s")
            y2s = wp.tile([128, 4, NTOK], f16, tag="y2s")
            h0 = wp.tile([128, 4, 16], f16, tag="h0")
            nc.vector.memset(h0[:], 0.0)
            pooled = wp.tile([128, 4, 16], f16, tag="pooled")
            nc.vector.memset(pooled[:], -60000.0)

            ring = {}

            # ---------- GEMM item generators (finely sliced side work) ----
            # Each 256-col xp block is computed straight into an SBUF ring
            # tile; items are individually small so they pace between steps.
            # Bias is folded into the GEMM via a K=1 bias-row matmul, so the
            # PSUM->SBUF downcast is a bias-free Copy fused over 2 m-chunks
            # (fewer, Act-only fin ops keep the DVE queue clear for chains).
            def gemm_items_A(cid, eT_dram, w_sb, blk):
                sl = slice(blk * 256, (blk + 1) * 256)
                st = {}
                items = []

                def open_():
                    ring[(cid, blk)] = xpb.tile(
                        [128, 12, 256], f16, tag="xpb", name="xpblk")
                    et = io.tile([128, 3, 256], f16, tag="et_in")
                    nc.sync.dma_start(out=et[:], in_=eT_dram[:, :, sl])
                    st["et"] = et

                items.append(open_)
                for m2 in range(6):
                    def mk(m2):
                        def mmsa():
                            p = ps.tile([128, 2, 256], f32, tag="gps")
                            st[m2] = p
                            m = 2 * m2
                            for k in range(3):
                                nc.tensor.matmul(
                                    p[:, 0, :],
                                    w_sb[:, k, m * 128 : (m + 1) * 128],
                                    st["et"][:, k, :],
                                    start=(k == 0), stop=(k == 2),
                                )

                        def mmsb():
                            p = st[m2]
                            m = 2 * m2 + 1
                            for k in range(3):
                                nc.tensor.matmul(
                                    p[:, 1, :],
                                    w_sb[:, k, m * 128 : (m + 1) * 128],
                                    st["et"][:, k, :],
                                    start=(k == 0), stop=(k == 2),
                                )

                        def fin():
                            p = st.pop(m2)
                            xb = ring[(cid, blk)]
                            nc.scalar.activation(
                                xb[:, 2 * m2 : 2 * m2 + 2, :], p[:], AF.Copy,
                            )

                        return mmsa, mmsb, fin

                    items += list(mk(m2))
                return items

            def gemm_items_L(blk):
                sl = slice(blk * 256, (blk + 1) * 256)
                st = {}
                items = []

                def open_():
                    ring[("L", blk)] = xpb.tile(
                        [128, 12, 256], f16, tag="xpb", name="xpblk")

                items.append(open_)
                for m2 in range(6):
                    def mk(m2):
                        def half(m, pi, p):
                            for k in range(4):
                                nc.tensor.matmul(
                                    p[:, pi, :],
                                    wa_s[:, k, m * 128 : (m + 1) * 128],
                                    y1s[:, k, sl],
                                    start=(k == 0), stop=False,
                                )
                            for k in range(4):
                                nc.tensor.matmul(
                                    p[:, pi, :],
                                    wb_s[:, k, m * 128 : (m + 1) * 128],
                                    y2s[:, k, sl],
                                    start=False, stop=False,
                                )
                            nc.tensor.matmul(
                                p[:, pi, :], biasrL_s[0:1, m, :],
                                ones_s[0:1, :], start=False, stop=True,
                            )

                        def mmsa():
                            p = ps.tile([128, 2, 256], f32, tag="gps")
                            st[m2] = p
                            half(2 * m2, 0, p)

                        def mmsb():
                            half(2 * m2 + 1, 1, st[m2])

                        def fin():
                            p = st.pop(m2)
                            xb = ring[("L", blk)]
                            nc.scalar.activation(
                                xb[:, 2 * m2 : 2 * m2 + 2, :], p[:], AF.Copy,
                            )

                        return mmsa, mmsb, fin

                    items += list(mk(m2))
                return items

            # ---------- recurrence chain -----------------------------------
            class Chain:
                def __init__(self, cid, whh_sb, nbc_sb,
                             y=None, rev=False, use_pool=False,
                             split_sigma=False, qp_split=False):
                    self.cid = cid
                    self.whh = whh_sb
                    self.nbc = nbc_sb
                    self.y = y
                    self.rev = rev
                    self.use_pool = use_pool
                    self.split_sigma = split_sigma
                    self.qp_split = qp_split
                    self.cur = None
                    self.hprev = h0
                    self.qprev = None
                    self.pprev = None

                def _col(self, t):
                    c = (T - 1 - t) if self.rev else t
                    return slice(c * 16, (c + 1) * 16)

                def step(self, t):
                    v = t % 16
                    if v == 0:
                        self.cur = ring.pop((self.cid, t // 16))
                    cur = self.cur
                    vs = slice(v * 16, (v + 1) * 16)
                    pn = psg.tile([128, 4, 16], f32, tag="pstn")
                    hp_ = self.hprev

                    def hrhs(k):
                        return (
                            hp_[:, k, :] if not isinstance(hp_, tuple)
                            else hp_[0][:, k, hp_[1]]
                        )

                    # One ungated injection matmul per PSUM tile opens that
                    # bank's single accumulation group (it runs ahead of the
                    # h-gated whh matmuls on the in-order PE); all whh
                    # matmuls then accumulate region-wise inside the open
                    # group, closing it on the last one.
                    def inj(dst, src):
                        nc.tensor.matmul(dst, ident_s[:], src,
                                         start=True, stop=False)

                    def whh_mms(dst, m0, m1, rhs=None, stop=True):
                        for m in range(m0, m1):
                            for k in range(4):
                                nc.tensor.matmul(
                                    dst[:, m - m0, :],
                                    self.whh[:, k, m * 128 : (m + 1) * 128],
                                    hrhs(k) if rhs is None else rhs[:, k, :],
                                    start=False,
                                    stop=(stop and m == m1 - 1 and k == 3),
                                )

                    if self.qp_split and self.split_sigma:
                        # gh = Whh*q + Whh*p (exact); the q half fires as
                        # soon as q(t-1) exists (mid-chain), only the p half
                        # gates the sigmoid -> h drops off the critical path.
                        pr = psg.tile([128, 4, 16], f32, tag="pstrz",
                                      name="pr")
                        pz = psg.tile([128, 4, 16], f32, tag="pstrz",
                                      name="pz")
                        if t == 0:
                            nc.tensor.matmul(pn[:], ident_s[:], self.nbc[:],
                                             start=True, stop=True)
                            nc.tensor.matmul(pr[:], ident_s[:],
                                             cur[:, 0:4, vs],
                                             start=True, stop=True)
                            nc.tensor.matmul(pz[:], ident_s[:],
                                             cur[:, 4:8, vs],
                                             start=True, stop=True)
                        else:
                            inj(pn[:], self.nbc[:])
                            inj(pr[:], cur[:, 0:4, vs])
                            inj(pz[:], cur[:, 4:8, vs])
                            qq, pp = self.qprev, self.pprev
                            whh_mms(pr, 0, 4, rhs=qq, stop=False)
                            whh_mms(pz, 4, 8, rhs=qq, stop=False)
                            whh_mms(pr, 0, 4, rhs=pp)
                            whh_mms(pz, 4, 8, rhs=pp)
                            whh_mms(pn, 8, 12)
                        rt = ew.tile([128, 4, 16], f32, tag=f"rt{self.cid}")
                        zt = ew.tile([128, 4, 16], f32, tag=f"zt{self.cid}")
                        nc.scalar.activation(rt[:], pr[:], AF.Sigmoid)
                        nc.scalar.activation(zt[:], pz[:], AF.Sigmoid)
                        rr, zz = rt[:], zt[:]
                    elif self.qp_split:
                        prz = psg.tile([128, 8, 16], f32, tag="pstrz",
                                       name="prz")
                        if t == 0:
                            nc.tensor.matmul(pn[:], ident_s[:], self.nbc[:],
                                             start=True, stop=True)
                            nc.tensor.matmul(prz[:], ident_s[:],
                                             cur[:, 0:8, vs],
                                             start=True, stop=True)
                        else:
                            inj(pn[:], self.nbc[:])
                            inj(prz[:], cur[:, 0:8, vs])
                            qq, pp = self.qprev, self.pprev
                            whh_mms(prz, 0, 8, rhs=qq, stop=False)
                            whh_mms(prz, 0, 8, rhs=pp)
                            whh_mms(pn, 8, 12)
                        rz = ew.tile([128, 8, 16], f32, tag=f"rz{self.cid}")
                        nc.scalar.activation(rz[:], prz[:], AF.Sigmoid)
                        rr, zz = rz[:, 0:4, :], rz[:, 4:8, :]
                    elif self.split_sigma:
                        inj(pn[:], self.nbc[:])
                        pr = psg.tile([128, 4, 16], f32, tag="pstrz",
                                      name="pr")
                        pz = psg.tile([128, 4, 16], f32, tag="pstrz",
                                      name="pz")
                        inj(pr[:], cur[:, 0:4, vs])
                        inj(pz[:], cur[:, 4:8, vs])
                        whh_mms(pr, 0, 4)
                        whh_mms(pz, 4, 8)
                        whh_mms(pn, 8, 12)
                        rt = ew.tile([128, 4, 16], f32, tag=f"rt{self.cid}")
                        zt = ew.tile([128, 4, 16], f32, tag=f"zt{self.cid}")
                        nc.scalar.activation(rt[:], pr[:], AF.Sigmoid)
                        nc.scalar.activation(zt[:], pz[:], AF.Sigmoid)
                        rr, zz = rt[:], zt[:]
                    else:
                        inj(pn[:], self.nbc[:])
                        prz = psg.tile([128, 8, 16], f32, tag="pstrz",
                                       name="prz")
                        inj(prz[:], cur[:, 0:8, vs])
                        whh_mms(prz, 0, 8)
                        whh_mms(pn, 8, 12)
                        rz = ew.tile([128, 8, 16], f32, tag=f"rz{self.cid}")
                        nc.scalar.activation(rz[:], prz[:], AF.Sigmoid)
                        rr, zz = rz[:, 0:4, :], rz[:, 4:8, :]
                    u = ew.tile([128, 4, 16], f32, tag=f"u{self.cid}")
                    nc.vector.scalar_tensor_tensor(
                        out=u[:], in0=pn[:], scalar=1.0,
                        in1=rr, op0=OP.mult, op1=OP.mult,
                    )
                    tn = ew.tile([128, 4, 16], f32, tag=f"tn{self.cid}")
                    nc.vector.scalar_tensor_tensor(
                        out=tn[:], in0=u[:], scalar=1.0,
                        in1=cur[:, 8:12, vs], op0=OP.mult, op1=OP.add,
                    )
                    nn = ew.tile([128, 4, 16], f16, tag=f"nn{self.cid}")
                    nc.scalar.activation(nn[:], tn[:], AF.Tanh, scale=-1.0)
                    q = ew.tile([128, 4, 16], f16, tag=f"q{self.cid}")
                    nc.vector.scalar_tensor_tensor(
                        out=q[:], in0=zz, scalar=1.0,
                        in1=hp_[:] if not isinstance(hp_, tuple)
                        else hp_[0][:, :, hp_[1]],
                        op0=OP.mult, op1=OP.mult,
                    )
                    p = ew.tile([128, 4, 16], f16, tag=f"p{self.cid}")
                    nc.vector.scalar_tensor_tensor(
                        out=p[:], in0=zz, scalar=1.0,
                        in1=nn[:], op0=OP.subtract, op1=OP.mult,
                    )
                    if self.y is not None:
                        cs = self._col(t)
                        nc.vector.scalar_tensor_tensor(
                            out=self.y[:, :, cs], in0=q[:], scalar=1.0,
                            in1=p[:], op0=OP.mult, op1=OP.add,
                        )
                        self.hprev = (self.y, cs)
                    else:
                        hn = hp.tile([128, 4, 16], f16, tag="hL")
                        nc.vector.scalar_tensor_tensor(
                            out=hn[:], in0=q[:], scalar=1.0,
                            in1=p[:], op0=OP.mult, op1=OP.add,
                        )
                        if self.use_pool:
                            nc.vector.scalar_tensor_tensor(
                                out=pooled[:], in0=hn[:], scalar=1.0,
                                in1=pooled[:], op0=OP.mult, op1=OP.max,
                            )
                        self.hprev = hn
                    self.qprev = q
                    self.pprev = p

            # ---------- item pacing ---------------------------------------
            def paced(items, t0, t1):
                n = len(items)
                w = max(1, t1 - t0)
                return [(t0 + (i * w) // n, fn) for i, fn in enumerate(items)]

            def run_phase(chains, sched):
                sched = sorted(sched, key=lambda x: x[0])
                si = 0
                for t in range(T):
                    while si < len(sched) and sched[si][0] <= t:
                        sched[si][1]()
                        si += 1
                    for c in chains:
                        c.step(t)
                while si < len(sched):
                    sched[si][1]()
                    si += 1

            # ---------- phase A: L0 fwd + bwd interleaved -----------------
            for fn in gemm_items_A("A", eT1, wih1_s, 0):
                fn()
            wih2_s = load(wih2, [128, 3, G])
            for fn in gemm_items_A("B", eT2, wih2_s, 0):
                fn()
            ident_s = load(ident, [128, 128])
            nbc1_s = load(nbc1, [128, 4, 16])
            nbc2_s = load(nbc2, [128, 4, 16])
            ones_s = load(ones, [1, 256])
            whh1_s = load(whh1, [128, 4, G])
            whh2_s = load(whh2, [128, 4, G])
            chA = Chain("A", whh1_s, nbc1_s, y=y1s, rev=False,
                        qp_split=True)
            chB = Chain("B", whh2_s, nbc2_s, y=y2s, rev=True,
                        qp_split=True)
            schedA = []
            for c in range(1, NBLK):
                schedA += paced(
                    gemm_items_A("A", eT1, wih1_s, c)
                    + gemm_items_A("B", eT2, wih2_s, c),
                    max(0, 16 * c - 44), 16 * c - 8,
                )
            run_phase([chA, chB], schedA)

            # ---------- phase B: L1 (merged xpL GEMM) ---------------------
            for fn in gemm_items_L(0):
                fn()
            chL = Chain("L", whhL_s, nbcL_s, use_pool=True,
                        split_sigma=True, qp_split=True)
            schedB = []
            for c in range(1, NBLK):
                schedB += paced(
                    gemm_items_L(c), max(0, 16 * c - 44), 16 * c - 8
                )
            run_phase([chL], schedB)

            # ---------- head ----------------------------------------------
            hd = psg.tile([128, 4, 16], f32, tag="pstn")
            for k in range(4):
                nc.tensor.matmul(
                    hd[:, 0, :], w1h_s[:, k, :], pooled[:, k, :],
                    start=(k == 0), stop=(k == 3),
                )
            ho = io.tile([128, 16], f32, tag="ho")
            nc.vector.tensor_copy(out=ho[:], in_=hd[:, 0, :])
            nc.sync.dma_start(out=headout[:], in_=ho[:])
            if DBG:
                nc.sync.dma_start(out=y1o[:], in_=y1s[:])
                nc.sync.dma_start(out=y2o[:], in_=y2s[:])
                nc.sync.dma_start(out=pooledo[:], in_=pooled[:])

    _split_multiwaits(nc, mybir)
    try:
        ents = getattr(tc, "_perfetto_entries", None)
        span = None
        if ents:
            starts = [e[1] for e in ents if e[1] is not None]
            ends = [e[2] if e[2] is not None else e[1] for e in ents]
            if starts and ends:
                span = int(max(ends) - min(starts))
        _CACHE["model_ns"] = span
    except Exception:
        _CACHE["model_ns"] = None
    return nc


def _prep_core_inputs(inputs, g, role):
    """Host-side sharding/layout prep for core (batch group g, role)."""
    f16 = np.float16
    x = np.asarray(inputs["x"]).astype(np.int64)
    emb = np.asarray(inputs["emb"], dtype=np.float32)
    embp = np.zeros((V, EP), dtype=np.float32)
    embp[:, :E] = emb
    embp[:, E] = 1.0

    xg = x[g * BL : (g + 1) * BL]
    e = embp[xg]
    eT_f = np.ascontiguousarray(e.transpose(2, 1, 0).reshape(EP, NTOK))
    er = e[:, ::-1, :]
    eT_r = np.ascontiguousarray(er.transpose(2, 1, 0).reshape(EP, NTOK))

    def ktile(wT, kt):
        Kd, Gd = wT.shape
        assert Kd == kt * 128
        return np.ascontiguousarray(
            wT.reshape(kt, 128, Gd).transpose(1, 0, 2)
        ).astype(f16)

    def e3(eT):
        return np.ascontiguousarray(
            eT.reshape(3, 128, NTOK).transpose(1, 0, 2)
        ).astype(f16)

    def biascols(b_ih, b_hh):
        bv = b_ih.copy()
        bv[: 2 * H] += b_hh[: 2 * H]
        rows = bv.reshape(1, 12, 128).astype(f16)
        nbc = np.repeat(
            np.ascontiguousarray(b_hh[2 * H :].reshape(4, 128).T)[:, :, None],
            16, axis=2,
        ).astype(f16)
        return rows, nbc

    w_ih0 = np.asarray(inputs["w_ih0"], dtype=np.float32)
    w_hh0 = np.asarray(inputs["w_hh0"], dtype=np.float32)
    b_ih0 = np.asarray(inputs["b_ih0"], dtype=np.float32)
    b_hh0 = np.asarray(inputs["b_hh0"], dtype=np.float32)
    w_ih1 = np.asarray(inputs["w_ih1"], dtype=np.float32)
    w_hh1 = np.asarray(inputs["w_hh1"], dtype=np.float32)
    b_ih1 = np.asarray(inputs["b_ih1"], dtype=np.float32)
    b_hh1 = np.asarray(inputs["b_hh1"], dtype=np.float32)
    w1 = np.asarray(inputs["w1"], dtype=np.float32)

    d1, d2 = (0, 1) if role == 0 else (1, 0)
    dL = role
    own_half = slice(0, H) if role == 0 else slice(H, 2 * H)
    oth_half = slice(H, 2 * H) if role == 0 else slice(0, H)

    def wihT(d, bv):
        w = np.zeros((G, EP), dtype=np.float32)
        w[:, :E] = w_ih0[d]
        w[:, E] = bv
        return ktile(w.T, 3)

    def biasvec(b_ih, b_hh):
        bv = b_ih.copy()
        bv[: 2 * H] += b_hh[: 2 * H]
        return bv

    bv1 = biasvec(b_ih0[d1], b_hh0[d1])
    bv2 = biasvec(b_ih0[d2], b_hh0[d2])
    _, n1c = biascols(b_ih0[d1], b_hh0[d1])
    _, n2c = biascols(b_ih0[d2], b_hh0[d2])
    bLc, nLc = biascols(b_ih1[dL], b_hh1[dL])

    m = {
        "eT1": e3(eT_f if role == 0 else eT_r),
        "eT2": e3(eT_r if role == 0 else eT_f),
        "wih1": wihT(d1, bv1),
        "wih2": wihT(d2, bv2),
        "whh1": ktile(w_hh0[d1].T, 4),
        "whh2": ktile(w_hh0[d2].T, 4),
        "wa": ktile(w_ih1[dL][:, own_half].T, 4),
        "wb": ktile(w_ih1[dL][:, oth_half].T, 4),
        "whhL": ktile(w_hh1[dL].T, 4),
        "biasrL": bLc,
        "ones": np.ones((1, 256), dtype=f16),
        "nbc1": n1c, "nbc2": n2c, "nbcL": nLc,
        "ident": np.eye(128, dtype=f16),
        "w1h": ktile(w1[:, own_half].T, 4),
    }
    return m


def kernel(**inputs) -> np.ndarray:
    from concourse.bass_utils import run_bass_kernel_spmd

    if "nc" not in _CACHE:
        _CACHE["nc"] = _build_nc()
    nc = _CACHE["nc"]

    core_ids = list(range(8))
    in_maps = []
    for c in core_ids:
        g, role = c % 4, c // 4
        in_maps.append(_prep_core_inputs(inputs, g, role))

    res = run_bass_kernel_spmd(nc, in_maps, core_ids)
    _CACHE["last_res"] = res

    b1 = np.asarray(inputs["b1"], dtype=np.float32)
    w2 = np.asarray(inputs["w2"], dtype=np.float32)
    b2 = np.asarray(inputs["b2"], dtype=np.float32)
    out = np.zeros((B, 2), dtype=np.float32)
    for g in range(4):
        p = (
            res.results[g]["headout"].astype(np.float32)
            + res.results[g + 4]["headout"].astype(np.float32)
        )
        hid = np.maximum(p + b1[:, None], 0.0)
        logits = w2 @ hid + b2[:, None]
        out[g * BL : (g + 1) * BL] = logits.T
    return out
